# revision 1
# baseline (speedup 1.0000x reference)
"""Trainium2 Bass kernel for PVT-style spatial-reduction attention.

Shapes (hardcoded): x [2, 4096, 256], HEAD=8, dh=32, SR=2, R=8, H=W=64.
Sharding: core c = (batch b = c//4, query block j = c%4). Each core computes
q/attention/proj for its 1024 query rows and redundantly computes the small
conv+LN+KV path for its batch (no collectives; the kernel is ScalarE-exp
bound, so the redundant PE work hides).

Layouts: "transposed activations" — channels on partitions, tokens on the
free dim. Weights are pre-transposed/cast to bf16 on the host. Per-core x is
pre-rotated on host so each core's own query block is rows 0:1024 (softmax
over KV tokens is permutation invariant, and the 2x2/stride-2 conv commutes
with 16-image-row rotations).

PSUM budget (8 banks): scores 2x[128,1024] (4) + pv 2x[128,512] (2) +
conv/proj lane 1x[128,512] (1) + stats/kv lane 1x[128,512] (1).
"""
import sys

if "/opt/trn_rl_repo" not in sys.path:
    sys.path.insert(0, "/opt/trn_rl_repo")

import numpy as np
import ml_dtypes

BF16NP = ml_dtypes.bfloat16

HEAD, DH, C, N, B, M, R = 8, 32, 256, 4096, 2, 1024, 8
NB = N // 4          # query rows per core
SCALE = DH ** -0.5
NCORES = 8
MAGIC = 0x5F3759DF

_CACHE = {}


def _build_program():
    import concourse.bass as bass
    import concourse.tile as tile
    from concourse.bacc import Bacc
    from concourse import mybir, masks

    F32 = mybir.dt.float32
    BF16 = mybir.dt.bfloat16
    I32 = mybir.dt.int32
    AF = mybir.ActivationFunctionType
    ALU = mybir.AluOpType

    nc = Bacc()
    P = 128
    S = 2          # kv strips
    ST = 512       # kv tokens per strip

    def bcast(ap, nparts):
        # partition-stride-0 broadcast of a 1-D DRAM AP
        return bass.AP(tensor=ap.tensor, offset=ap.offset,
                       ap=[[0, nparts]] + [list(d) for d in ap.ap])

    # ---- DRAM parameters (host-prepped layouts) ----
    xT_d = nc.declare_dram_parameter("xT", [P, 2, N], BF16, isOutput=False)
    qwT_d = nc.declare_dram_parameter("qwT", [P, 2, C], BF16, isOutput=False)
    kvwT_d = nc.declare_dram_parameter("kvwT", [P, 2, 2 * C], BF16, isOutput=False)
    pwT_d = nc.declare_dram_parameter("pwT", [P, 2, C], BF16, isOutput=False)
    srwT_d = nc.declare_dram_parameter("srwT", [P, 2, 4, C], BF16, isOutput=False)
    aqT_d = nc.declare_dram_parameter("aqT", [P, 2, R], BF16, isOutput=False)
    avT_d = nc.declare_dram_parameter("avT", [P, 2, R], BF16, isOutput=False)
    bqT_d = nc.declare_dram_parameter("bqT", [R, 2, P], BF16, isOutput=False)
    bvT_d = nc.declare_dram_parameter("bvT", [R, 2, P], BF16, isOutput=False)
    qb_d = nc.declare_dram_parameter("qb", [P, 2], F32, isOutput=False)
    srb_d = nc.declare_dram_parameter("srb", [P, 2], F32, isOutput=False)
    wg1_d = nc.declare_dram_parameter("wg1", [1, 4, P], BF16, isOutput=False)
    avg1_d = nc.declare_dram_parameter("avg1", [1, R], BF16, isOutput=False)
    pb_d = nc.declare_dram_parameter("pb", [C], F32, isOutput=False)
    out_d = nc.declare_dram_parameter("out", [NB, C], F32, isOutput=True)

    with tile.TileContext(nc) as tc:
        with tc.tile_pool(name="wgt", bufs=1) as WGT, \
             tc.tile_pool(name="acts", bufs=1) as ACTS, \
             tc.tile_pool(name="strips", bufs=2) as STR, \
             tc.tile_pool(name="tmp", bufs=3) as TMP, \
             tc.tile_pool(name="atn", bufs=2) as ATN, \
             tc.tile_pool(name="pt", bufs=32) as PT, \
             tc.tile_pool(name="fin", bufs=2) as FIN, \
             tc.tile_pool(name="big", bufs=2, space="PSUM") as PSB, \
             tc.tile_pool(name="pv", bufs=2, space="PSUM") as PSV, \
             tc.tile_pool(name="cv", bufs=1, space="PSUM") as PSC, \
             tc.tile_pool(name="kvl", bufs=1, space="PSUM") as PSK, \
             tc.tile_pool(name="dscr", bufs=1, space="DRAM") as DSCR:

            # ---------- weights ----------
            def wload(name, shape, dt, src):
                t = WGT.tile(shape, dt, tag=name)
                nc.sync.dma_start(out=t[:], in_=src)
                return t

            qwT = wload("qwT", [P, 2, C], BF16, qwT_d[:])
            kvwT = wload("kvwT", [P, 2, 2 * C], BF16, kvwT_d[:])
            pwT = wload("pwT", [P, 2, C], BF16, pwT_d[:])
            srwT = wload("srwT", [P, 2, 4, C], BF16, srwT_d[:])
            aqT = wload("aqT", [P, 2, R], BF16, aqT_d[:])
            avT = wload("avT", [P, 2, R], BF16, avT_d[:])
            bqT = wload("bqT", [R, 2, P], BF16, bqT_d[:])
            bvT = wload("bvT", [R, 2, P], BF16, bvT_d[:])
            qb = wload("qb", [P, 2], F32, qb_d[:])
            srb = wload("srb", [P, 2], F32, srb_d[:])
            wg1t = wload("wg1", [1, 4, P], BF16, wg1_d[:])
            avg1t = wload("avg1", [1, R], BF16, avg1_d[:])
            pbB = wload("pbB", [P, C], F32, bcast(pb_d.ap(), P))
            ones1 = WGT.tile([P, 1], BF16, tag="ones1")
            nc.gpsimd.memset(ones1[:], 1.0 / C)
            ident = WGT.tile([P, P], BF16, tag="ident")
            masks.make_identity(nc, ident[:])

            # persistent activations
            qT = ACTS.tile([P, 2, NB], BF16, tag="qT")
            outT = ACTS.tile([P, 2, NB], BF16, tag="outT")
            tq = ACTS.tile([R, NB], BF16, tag="tq")

            xTs, kts, vsb, ans, ascl = [], [], [], [], []

            # ---------- per-strip setup + q path ----------
            for s in range(S):
                xs_t = ACTS.tile([P, 2, 2048], BF16, tag=f"xT{s}")
                nc.gpsimd.dma_start(out=xs_t[:], in_=xT_d[:, :, s * 2048:(s + 1) * 2048])
                xTs.append(xs_t)

                # conv (2x2 stride-2 as 8 accumulated matmuls per out-chunk)
                xs_s = STR.tile([P, 2, ST], F32, tag="xs")
                for oc in range(2):
                    cps = PSC.tile([P, ST], F32, tag="cv")
                    first = True
                    for cc in range(2):
                        xv = xs_t[:, cc, :].rearrange(
                            "p (i a j b) -> p i a j b", i=16, a=2, j=32, b=2)
                        for di in range(2):
                            for dj in range(2):
                                nc.tensor.matmul(
                                    cps[:], srwT[:, cc, di * 2 + dj,
                                                 oc * P:(oc + 1) * P],
                                    xv[:, :, di, :, dj],
                                    start=first,
                                    stop=(cc == 1 and di == 1 and dj == 1))
                                first = False
                    nc.vector.tensor_scalar_add(
                        out=xs_s[:, oc, :], in0=cps[:], scalar1=srb[:, oc:oc + 1])

                # LN stats via (1/C)-ones matmul channel sums -> mean/E[x^2]
                xsb_s = STR.tile([P, 2, ST], BF16, tag="xsb")
                nc.gpsimd.tensor_copy(out=xsb_s[:], in_=xs_s[:])
                sq_s = STR.tile([P, 2, ST], BF16, tag="sq")
                nc.vector.tensor_mul(out=sq_s[:], in0=xsb_s[:], in1=xsb_s[:])
                sxp = PSK.tile([1, ST], F32, tag="kvl")
                nc.tensor.matmul(sxp[:], ones1[:], xsb_s[:, 0, :], start=True, stop=False)
                nc.tensor.matmul(sxp[:], ones1[:], xsb_s[:, 1, :], start=False, stop=True)
                negmu = TMP.tile([1, ST], BF16, tag="negmu")
                nc.vector.tensor_scalar_mul(out=negmu[:], in0=sxp[:], scalar1=-1.0)
                sxxp = PSK.tile([1, ST], F32, tag="kvl")
                nc.tensor.matmul(sxxp[:], ones1[:], sq_s[:, 0, :], start=True, stop=False)
                nc.tensor.matmul(sxxp[:], ones1[:], sq_s[:, 1, :], start=False, stop=True)
                ex2_sb = TMP.tile([1, ST], F32, tag="ex2sb")
                nc.vector.tensor_copy(out=ex2_sb[:], in_=sxxp[:])
                # chunk-major repack [1, 512] -> [128, 4]  (t = g*128 + p)
                # via DRAM bounce (SBUF source APs can't express the permute)
                nm_d = DSCR.tile([ST], BF16, tag=f"nm{s}")
                nc.sync.dma_start(out=nm_d[:], in_=negmu[:])
                ex_d = DSCR.tile([ST], F32, tag=f"ex{s}")
                nc.sync.dma_start(out=ex_d[:], in_=ex2_sb[:])
                mur = TMP.tile([P, 4], BF16, tag="mur")
                nc.sync.dma_start(out=mur[:],
                                  in_=nm_d[:].rearrange("(g p) -> p g", p=P))
                ex2r = TMP.tile([P, 4], F32, tag="ex2r")
                nc.sync.dma_start(out=ex2r[:],
                                  in_=ex_d[:].rearrange("(g p) -> p g", p=P))
                # rstd via quake rsqrt (1 newton); an = rstd, ascl = SCALE*rstd
                nmu2 = TMP.tile([P, 4], F32, tag="nmu2")
                nc.vector.scalar_tensor_tensor(out=nmu2[:], in0=mur[:], scalar=-1.0,
                                               in1=mur[:], op0=ALU.mult, op1=ALU.mult)
                ve = TMP.tile([P, 4], F32, tag="ve")
                nc.vector.scalar_tensor_tensor(out=ve[:], in0=nmu2[:], scalar=1e-5,
                                               in1=ex2r[:], op0=ALU.add, op1=ALU.add)
                hsh = TMP.tile([P, 4], I32, tag="hsh")
                nc.vector.tensor_scalar(out=hsh[:], in0=ve[:].bitcast(I32), scalar1=1,
                                        scalar2=None, op0=ALU.logical_shift_right)
                nc.vector.tensor_scalar(out=hsh[:], in0=hsh[:], scalar1=-1,
                                        scalar2=MAGIC, op0=ALU.mult, op1=ALU.add)
                y0 = hsh[:].bitcast(F32)
                nt = TMP.tile([P, 4], F32, tag="nt")
                nc.vector.tensor_mul(out=nt[:], in0=y0, in1=y0)
                nc.vector.scalar_tensor_tensor(out=nt[:], in0=nt[:], scalar=-0.5,
                                               in1=ve[:], op0=ALU.mult, op1=ALU.mult)
                nc.vector.tensor_scalar_add(out=nt[:], in0=nt[:], scalar1=1.5)
                an_s = STR.tile([P, 4], F32, tag="an")
                nc.vector.tensor_mul(out=an_s[:], in0=y0, in1=nt[:])
                ascl_s = STR.tile([P, 4], F32, tag="ascl")
                nc.vector.tensor_scalar_mul(out=ascl_s[:], in0=an_s[:], scalar1=SCALE)
                ans.append(an_s)
                ascl.append(ascl_s)

                # shared lora for k and v: t2raw = Avg @ xs_raw - mu*avg1
                t2p = PSK.tile([R, ST], F32, tag="kvl")
                nc.tensor.matmul(t2p[:], avT[:, 0, :], xsb_s[:, 0, :], start=True, stop=False)
                nc.tensor.matmul(t2p[:], avT[:, 1, :], xsb_s[:, 1, :], start=False, stop=False)
                nc.tensor.matmul(t2p[:], avg1t[:], negmu[:], start=False, stop=True)
                t2 = TMP.tile([R, ST], BF16, tag="t2")
                nc.vector.tensor_copy(out=t2[:], in_=t2p[:])

                kts_s = STR.tile([P, 2, ST], BF16, tag="kts")
                vtmp_s = STR.tile([P, 2, ST], BF16, tag="vtmp")
                for kvoc in range(4):
                    kps = PSK.tile([P, ST], F32, tag="kvl")
                    nc.tensor.matmul(kps[:], kvwT[:, 0, kvoc * P:(kvoc + 1) * P],
                                     xsb_s[:, 0, :], start=True, stop=False)
                    nc.tensor.matmul(kps[:], kvwT[:, 1, kvoc * P:(kvoc + 1) * P],
                                     xsb_s[:, 1, :], start=False, stop=False)
                    nc.tensor.matmul(kps[:], wg1t[:, kvoc, :], negmu[:],
                                     start=False, stop=False)
                    nc.tensor.matmul(kps[:], bvT[:, kvoc % 2, :], t2[:],
                                     start=False, stop=True)
                    dst = kts_s[:, kvoc, :] if kvoc < 2 else vtmp_s[:, kvoc - 2, :]
                    nc.vector.tensor_copy(out=dst, in_=kps[:])
                kts.append(kts_s)

                # v transpose to [m, c] (PE transpose) + ones column
                vsb_s = STR.tile([P, 4, HEAD, DH + 1], BF16, tag="vsb")
                for vc in range(2):
                    for u4 in range(4):
                        vtp = PSK.tile([P, P], BF16, tag="kvl")
                        nc.tensor.transpose(vtp[:],
                                            vtmp_s[:, vc, u4 * P:(u4 + 1) * P],
                                            ident[:])
                        nc.vector.tensor_scalar_mul(
                            out=vsb_s[:, u4, vc * 4:(vc + 1) * 4, 0:DH],
                            in0=vtp[:].rearrange("p (h d) -> p h d", d=DH),
                            scalar1=an_s[:, u4:u4 + 1])
                nc.gpsimd.memset(vsb_s[:, :, :, DH:DH + 1], 1.0)
                vsb.append(vsb_s)

                if s == 0:
                    # q path (only needs x rows 0:1024 = first half of strip 0)
                    tqp = PSB.tile([R, NB], F32, tag="big")
                    for nh in range(2):
                        sl = slice(nh * 512, (nh + 1) * 512)
                        nc.tensor.matmul(tqp[:, sl], aqT[:, 0, :], xs_t[:, 0, sl],
                                         start=True, stop=False)
                        nc.tensor.matmul(tqp[:, sl], aqT[:, 1, :], xs_t[:, 1, sl],
                                         start=False, stop=True)
                    nc.vector.tensor_copy(out=tq[:], in_=tqp[:])
                    for oc in range(2):
                        qps = PSB.tile([P, NB], F32, tag="big")
                        for nh in range(2):
                            sl = slice(nh * 512, (nh + 1) * 512)
                            nc.tensor.matmul(qps[:, sl],
                                             qwT[:, 0, oc * P:(oc + 1) * P],
                                             xs_t[:, 0, sl], start=True, stop=False)
                            nc.tensor.matmul(qps[:, sl],
                                             qwT[:, 1, oc * P:(oc + 1) * P],
                                             xs_t[:, 1, sl], start=False, stop=False)
                            nc.tensor.matmul(qps[:, sl], bqT[:, oc, :], tq[:, sl],
                                             start=False, stop=True)
                        nc.vector.tensor_scalar_add(
                            out=qT[:, oc, :], in0=qps[:], scalar1=qb[:, oc:oc + 1])


            # ---------- attention: 4 head pairs, software-pipelined ----------
            def emit_scores(g, mc, pts):
                ch, r0 = g // 2, 64 * (g % 2)
                s, ml = mc // 4, mc % 4
                for h01 in range(2):
                    rr = r0 + 32 * h01
                    stile = PSB.tile([P, NB], F32, tag="big")
                    lhsT = kts[s][rr:rr + 32, ch, ml * P:(ml + 1) * P]
                    for nh in range(2):
                        sl = slice(nh * 512, (nh + 1) * 512)
                        nc.tensor.matmul(stile[:, sl], lhsT,
                                         qT[rr:rr + 32, ch, sl],
                                         start=True, stop=True,
                                         tile_position=(rr, 0))
                    pt_t = PT.tile([P, NB], BF16, tag="pt")
                    nc.scalar.activation(out=pt_t[:], in_=stile[:],
                                         func=AF.Exp,
                                         scale=ascl[s][:, ml:ml + 1])
                    pts[(h01, mc)] = pt_t

            def pv_mm(g, nh, pvp, pts, mc):
                sl = slice(nh * 512, (nh + 1) * 512)
                s, ml = mc // 4, mc % 4
                for h01 in range(2):
                    h = 2 * g + h01
                    nc.tensor.matmul(
                        pvp[64 * h01:64 * h01 + DH + 1, :],
                        vsb[s][:, ml, h, :], pts[(h01, mc)][:, sl],
                        start=(mc == 0), stop=(mc == 7),
                        tile_position=(0, 64 * h01))

            def pv_tail(g, nh, pvp, rec, fac, tmpo, rec_s):
                ch, r0 = g // 2, 64 * (g % 2)
                sl = slice(nh * 512, (nh + 1) * 512)
                # softmax denominators -> factors (DRAM-bounce broadcast)
                nc.vector.reciprocal(out=rec[0:1, sl], in_=pvp[DH:DH + 1, :])
                nc.vector.reciprocal(out=rec[32:33, sl], in_=pvp[64 + DH:64 + DH + 1, :])
                nc.sync.dma_start(out=rec_s[0, sl], in_=rec[0:1, sl])
                nc.sync.dma_start(out=rec_s[1, sl], in_=rec[32:33, sl])
                for h01 in range(2):
                    nc.sync.dma_start(out=fac[64 * h01:64 * h01 + DH, sl],
                                      in_=bcast(rec_s[h01, sl], DH))
                for h01 in range(2):
                    nc.vector.tensor_mul(out=tmpo[64 * h01:64 * h01 + DH, sl],
                                         in0=pvp[64 * h01:64 * h01 + DH, :],
                                         in1=fac[64 * h01:64 * h01 + DH, sl])
                    nc.scalar.dma_start(
                        out=outT[r0 + 32 * h01:r0 + 32 * h01 + 32, ch, sl],
                        in_=tmpo[64 * h01:64 * h01 + DH, sl])

            for g in range(4):
                pts = {}
                rec = ATN.tile([33, NB], F32, tag="rec")
                fac = ATN.tile([P, NB], F32, tag="fac")
                tmpo = ATN.tile([P, NB], BF16, tag="tmpo")
                rec_s = DSCR.tile([2, NB], F32, tag=f"rec{g}")
                pvp0 = PSV.tile([P, 512], F32, tag="pv")
                pvp1 = PSV.tile([P, 512], F32, tag="pv")
                for mc in range(8):
                    emit_scores(g, mc, pts)
                    pv_mm(g, 0, pvp0, pts, mc)
                    pv_mm(g, 1, pvp1, pts, mc)
                pv_tail(g, 0, pvp0, rec, fac, tmpo, rec_s)
                pv_tail(g, 1, pvp1, rec, fac, tmpo, rec_s)

            # ---------- output projection ----------
            for t8 in range(8):
                pp = PSC.tile([P, C], F32, tag="cv")
                nc.tensor.matmul(pp[:], outT[:, 0, t8 * P:(t8 + 1) * P],
                                 pwT[:, 0, :], start=True, stop=False)
                nc.tensor.matmul(pp[:], outT[:, 1, t8 * P:(t8 + 1) * P],
                                 pwT[:, 1, :], start=False, stop=True)
                fin = FIN.tile([P, C], F32, tag="fin")
                nc.vector.tensor_add(out=fin[:], in0=pp[:], in1=pbB[:])
                nc.scalar.dma_start(out=out_d[t8 * P:(t8 + 1) * P, :], in_=fin[:])

    nc.finalize()
    return nc


def _prep_shared(q_w, q_b, kv_w, kv_b, proj_w, proj_b, a_q, b_q, a_v, b_v,
                 sr_w, sr_b, ln_g, ln_b):
    f32 = np.float32

    def chunkT(w):  # [in, out] -> [128, n_in_chunks, out]
        wt = np.ascontiguousarray(np.asarray(w, f32).T)
        ic, oc = wt.shape
        return np.ascontiguousarray(
            wt.reshape(ic // 128, 128, oc).transpose(1, 0, 2)).astype(BF16NP)

    def pcols(v):  # [n*128] -> [128, n]
        v = np.asarray(v, f32)
        return np.ascontiguousarray(v.reshape(-1, 128).T)

    kv_w = np.asarray(kv_w, f32)
    a_v = np.asarray(a_v, f32)
    b_v = np.asarray(b_v, f32)
    g = np.asarray(ln_g, f32)
    bb = np.asarray(ln_b, f32)
    proj_w = np.asarray(proj_w, f32)
    # fold LayerNorm gamma into kv/a_v weights; mean via rank-1 correction;
    # k-side constants dropped (softmax shift invariance), v-side constants
    # folded into the projection bias.
    Wg = kv_w * g[None, :]
    wg1 = Wg.sum(1)
    Avg = a_v * g[None, :]
    avg1 = Avg.sum(1)
    wbt = kv_w @ bb + np.asarray(kv_b, f32)
    dconst = b_v @ (a_v @ bb)
    wv_const = wbt[C:] + dconst
    pb_eff = np.asarray(proj_b, f32) + proj_w @ wv_const

    srwT = np.asarray(sr_w, f32).transpose(1, 2, 3, 0).reshape(2, 128, 4, C)
    srwT = np.ascontiguousarray(srwT.transpose(1, 0, 2, 3)).astype(BF16NP)
    bqT = np.ascontiguousarray(np.asarray(b_q, f32).T.reshape(R, 2, 128)).astype(BF16NP)
    bvT = np.ascontiguousarray(b_v.T.reshape(R, 2, 128)).astype(BF16NP)
    return dict(
        qwT=chunkT(q_w), kvwT=chunkT(Wg), pwT=chunkT(proj_w),
        srwT=srwT, aqT=chunkT(a_q), avT=chunkT(Avg), bqT=bqT, bvT=bvT,
        qb=pcols(q_b), srb=pcols(sr_b),
        wg1=np.ascontiguousarray(wg1.reshape(1, 4, 128)).astype(BF16NP),
        avg1=np.ascontiguousarray(avg1.reshape(1, R)).astype(BF16NP),
        pb=pb_eff,
    )


def kernel(x, q_w, q_b, kv_w, kv_b, proj_w, proj_b, a_q, b_q, a_v, b_v,
           sr_w, sr_b, ln_g, ln_b, H, W):
    from concourse.bass_utils import run_bass_kernel_spmd

    x = np.asarray(x, np.float32)
    assert x.shape == (B, N, C) and int(H) == 64 and int(W) == 64

    if "nc" not in _CACHE:
        _CACHE["nc"] = _build_program()
    nc = _CACHE["nc"]

    shared = _prep_shared(q_w, q_b, kv_w, kv_b, proj_w, proj_b, a_q, b_q,
                          a_v, b_v, sr_w, sr_b, ln_g, ln_b)
    in_maps = []
    for c in range(NCORES):
        b, j = c // 4, c % 4
        xb = np.roll(x[b], -NB * j, axis=0)          # own block at rows 0:1024
        xT = np.ascontiguousarray(xb.T.astype(BF16NP))  # [256, 4096]
        xT = np.ascontiguousarray(
            xT.reshape(2, 128, N).transpose(1, 0, 2))   # [128, 2, 4096]
        in_maps.append(dict(shared, xT=xT))

    res = run_bass_kernel_spmd(nc, in_maps, list(range(NCORES)))
    out = np.empty((B, N, C), np.float32)
    for c in range(NCORES):
        b, j = c // 4, c % 4
        out[b, NB * j:NB * (j + 1)] = res.results[c]["out"]
    return out



# revision 16
# speedup vs baseline: 1.2373x; 1.2373x over previous
"""Trainium2 Bass kernel for PVT-style spatial-reduction attention (v2).

Shapes (hardcoded): x [2, 4096, 256], HEAD=8, dh=32, SR=2, R=8, H=W=64.
Sharding: core c = (batch b = c//4, query block j = c%4). Each core computes
q/attention/proj for its 1024 query rows and redundantly computes the small
conv+LN+KV path for its batch. Per-core x is pre-rotated on host so each
core's own query block is rows 0:1024.

v2 design vs baseline:
- fp16 compute chain (conv/kv/q/pv/proj) instead of bf16 (8x less quant err).
- Scores via fp8e4m3 DoubleRow matmuls (0.5 cyc/row): q/k projections emit a
  [16lane x 2half] per-head layout via host-permuted weight columns; packed
  fp8 copies are SWDGE-repacked to 32-aligned head bases.
- Transposed pv: softmax numerators/denominators computed with pts as the
  stationary operand -> pv output is [q, dh+1] (33 rows/matmul vs 512), and
  denominators land per-partition (reciprocal+scale, no DMA broadcast).
- Softmax exp split across ScalarE (true Exp) and DVE (one-op Schraudolph
  int16 bit-trick into fp16, rms ~1.6%).
- Few, large DMAs (weight blob, 2 stat bounces/strip) - HWDGE trigger is a
  serialized ~630ns global slot in the cost model.

PSUM (8 banks): P1 2x[128,1024]f32 (4) for q-path/scores/proj, CVKV
2x[128,512]f32 (2) for conv/kv/transposes, PV 2x[128,8,33]f32 (2) for pv.
"""
import sys

if "/opt/trn_rl_repo" not in sys.path:
    sys.path.insert(0, "/opt/trn_rl_repo")

import numpy as np

F16NP = np.float16

HEAD, DH, C, N, B, M, R = 8, 32, 256, 4096, 2, 1024, 8
NB = N // 4          # query rows per core
SCALE = DH ** -0.5
NCORES = 8
MAGIC = 0x5F3759DF
LOG2E8 = 1477.3195879  # 2^10 / ln 2
BITB = 15317.95        # tuned fp16 Schraudolph offset (trunc semantics)

# per-head exp engine schedule over mc=0..7 (A=ScalarE true exp, D=DVE bit trick)
EXP_SCHED = "AADAADAD"

# weight blob column offsets (fp16 elements)
O_QWT, O_KVWT, O_PWT, O_SRWT = 0, 512, 1536, 2048
O_AQT, O_AVT, O_IDENT, O_BQT = 4096, 4112, 4128, 4256
O_BVTK, O_BVTV, O_WG1K, O_WG1V = 4512, 4768, 5024, 5280
O_AVG1, O_ONE1C = 5536, 5544
BLOB_COLS = 5552

_CACHE = {}
DEBUG = False


def _build_program():
    import concourse.bass as bass
    import concourse.tile as tile
    from concourse.bacc import Bacc
    from concourse import mybir, masks

    F32 = mybir.dt.float32
    F16 = mybir.dt.float16
    FP8 = mybir.dt.float8e4
    I16 = mybir.dt.int16
    I32 = mybir.dt.int32
    AF = mybir.ActivationFunctionType
    ALU = mybir.AluOpType
    PM = mybir.MatmulPerfMode

    nc = Bacc()
    P = 128
    S = 2          # strips
    ST = 512       # kv tokens per strip

    def s0(ap, n):
        # stride-0 broadcast along a new innermost free dim
        return bass.AP(tensor=ap.tensor, offset=ap.offset,
                       ap=[list(d) for d in ap.ap] + [[0, n]])

    def bcast(ap, nparts):
        return bass.AP(tensor=ap.tensor, offset=ap.offset,
                       ap=[[0, nparts]] + [list(d) for d in ap.ap])

    U8 = mybir.dt.uint8
    xT_d = nc.declare_dram_parameter("xT", [P, 2, N], F16, isOutput=False)
    wb_d = nc.declare_dram_parameter("wb", [P, BLOB_COLS], F16, isOutput=False)
    wf_d = nc.declare_dram_parameter("wf", [P, 4], F32, isOutput=False)
    pb_d = nc.declare_dram_parameter("pb", [C], F32, isOutput=False)
    out_d = nc.declare_dram_parameter("out", [NB, C], F32, isOutput=True)
    if DEBUG:
        dbg = {
            "d_xsb": nc.declare_dram_parameter("d_xsb", [2, P, 2, 512], F16, isOutput=True),
            "d_an": nc.declare_dram_parameter("d_an", [2, P, 4], F32, isOutput=True),
            "d_k8": nc.declare_dram_parameter("d_k8", [P, 2, M], U8, isOutput=True),
            "d_q8": nc.declare_dram_parameter("d_q8", [P, 2, NB], U8, isOutput=True),
            "d_kA": nc.declare_dram_parameter("d_kA", [P, 2, M], U8, isOutput=True),
            "d_qA": nc.declare_dram_parameter("d_qA", [P, 2, NB], U8, isOutput=True),
            "d_vsb": nc.declare_dram_parameter("d_vsb", [2, P, 4, HEAD, DH + 1], F16, isOutput=True),
            "d_pts0": nc.declare_dram_parameter("d_pts0", [P, 8, NB], F16, isOutput=True),
            "d_rec": nc.declare_dram_parameter("d_rec", [HEAD, P, 8], F32, isOutput=True),
            "d_onorm": nc.declare_dram_parameter("d_onorm", [P, 8, HEAD, DH], F16, isOutput=True),
            "d_outT": nc.declare_dram_parameter("d_outT", [P, 2, NB], F16, isOutput=True),
        }

    with tile.TileContext(nc) as tc:
        with tc.tile_pool(name="wgt", bufs=1) as WGT, \
             tc.tile_pool(name="acts", bufs=1) as ACTS, \
             tc.tile_pool(name="str", bufs=2) as STR, \
             tc.tile_pool(name="tmp", bufs=2) as TMP, \
             tc.tile_pool(name="pts", bufs=2) as PTS, \
             tc.tile_pool(name="fin", bufs=2) as FIN, \
             tc.tile_pool(name="p1", bufs=2, space="PSUM") as P1, \
             tc.tile_pool(name="cvkv", bufs=2, space="PSUM") as CVKV, \
             tc.tile_pool(name="pv", bufs=2, space="PSUM") as PV, \
             tc.tile_pool(name="dscr", bufs=2, space="DRAM") as DSCR:

            # ---------------- loads ----------------
            wb = WGT.tile([P, BLOB_COLS], F16, tag="wb")
            nc.sync.dma_start(out=wb[:], in_=wb_d[:])
            wf = WGT.tile([P, 4], F32, tag="wf")
            nc.sync.dma_start(out=wf[:], in_=wf_d[:])
            pbB = WGT.tile([P, C], F32, tag="pbB")
            nc.sync.dma_start(out=pbB[:], in_=bcast(pb_d.ap(), P))

            qwT = wb[:, O_QWT:O_QWT + 512].rearrange("p (ch c) -> p ch c", ch=2)
            kvwT = wb[:, O_KVWT:O_KVWT + 1024].rearrange("p (ch c) -> p ch c", ch=2)
            pwT = wb[:, O_PWT:O_PWT + 512].rearrange("p (ch c) -> p ch c", ch=2)
            srwT = wb[:, O_SRWT:O_SRWT + 2048].rearrange(
                "p (ch t c) -> p ch t c", ch=2, t=4)
            aqT = wb[:, O_AQT:O_AQT + 16].rearrange("p (ch r) -> p ch r", ch=2)
            avT = wb[:, O_AVT:O_AVT + 16].rearrange("p (ch r) -> p ch r", ch=2)
            ident = wb[:, O_IDENT:O_IDENT + 128]
            bqT = wb[0:R, O_BQT:O_BQT + 256].rearrange("r (ch c) -> r ch c", ch=2)
            bvTk = wb[0:R, O_BVTK:O_BVTK + 256].rearrange("r (ch c) -> r ch c", ch=2)
            bvTv = wb[0:R, O_BVTV:O_BVTV + 256].rearrange("r (ch c) -> r ch c", ch=2)
            wg1k = wb[0:1, O_WG1K:O_WG1K + 256].rearrange("a (ch c) -> a ch c", ch=2)
            wg1v = wb[0:1, O_WG1V:O_WG1V + 256].rearrange("a (ch c) -> a ch c", ch=2)
            avg1 = wb[0:1, O_AVG1:O_AVG1 + 8]
            ones1c = wb[:, O_ONE1C:O_ONE1C + 1]

            # persistent activations
            k8tmp = ACTS.tile([P, 2, M], FP8, tag="k8tmp")
            q8tmp = ACTS.tile([P, 2, NB], FP8, tag="q8tmp")
            kA = ACTS.tile([P, 2, M], FP8, tag="kA")
            kB = ACTS.tile([P, 2, M], FP8, tag="kB")
            qA = ACTS.tile([P, 2, NB], FP8, tag="qA")
            qB = ACTS.tile([P, 2, NB], FP8, tag="qB")
            outT = ACTS.tile([P, 2, NB], F16, tag="outT")
            onorm = ACTS.tile([P, 8, HEAD, DH], F16, tag="onorm")
            tq = ACTS.tile([R, NB], F16, tag="tq")

            xTs, vsbs, ascls, abits = [], [], [], []

            # ---------------- per-strip conv/LN/KV ----------------
            for s in range(S):
                xs_t = ACTS.tile([P, 2, 2048], F16, tag=f"xT{s}")
                nc.gpsimd.dma_start(out=xs_t[:],
                                    in_=xT_d[:, :, s * 2048:(s + 1) * 2048])
                xTs.append(xs_t)

                xsb_s = STR.tile([P, 2, ST], F16, tag="xsb")
                for oc in range(2):
                    cps = CVKV.tile([P, ST], F32, tag="cv")
                    first = True
                    for cc in range(2):
                        xv = xs_t[:, cc, :].rearrange(
                            "p (i a j b) -> p i a j b", i=16, a=2, j=32, b=2)
                        for di in range(2):
                            for dj in range(2):
                                nc.tensor.matmul(
                                    cps[:], srwT[:, cc, di * 2 + dj,
                                                 oc * P:(oc + 1) * P],
                                    xv[:, :, di, :, dj],
                                    start=first,
                                    stop=(cc == 1 and di == 1 and dj == 1))
                                first = False
                    nc.vector.tensor_scalar_add(
                        out=xsb_s[:, oc, :], in0=cps[:], scalar1=wf[:, 2 + oc:3 + oc])

                if DEBUG:
                    nc.sync.dma_start(out=dbg["d_xsb"][s], in_=xsb_s[:])
                sq_s = STR.tile([P, 2, ST], F16, tag="sq")
                nc.gpsimd.tensor_mul(out=sq_s[:], in0=xsb_s[:], in1=xsb_s[:])

                # LN stats: channel sums via (1/C)-ones matmuls
                sxp = CVKV.tile([1, ST], F32, tag="cv")
                nc.tensor.matmul(sxp[:], ones1c, xsb_s[:, 0, :], start=True, stop=False)
                nc.tensor.matmul(sxp[:], ones1c, xsb_s[:, 1, :], start=False, stop=True)
                negmu = TMP.tile([1, ST], F16, tag="negmu")
                nc.vector.tensor_scalar_mul(out=negmu[:], in0=sxp[:], scalar1=-1.0)
                sxxp = CVKV.tile([1, ST], F32, tag="cv")
                nc.tensor.matmul(sxxp[:], ones1c, sq_s[:, 0, :], start=True, stop=False)
                nc.tensor.matmul(sxxp[:], ones1c, sq_s[:, 1, :], start=False, stop=True)
                ex2_sb = TMP.tile([1, ST], F32, tag="ex2sb")
                nc.vector.tensor_copy(out=ex2_sb[:], in_=sxxp[:])

                # chunk-major repack [1,512] -> [128,4] via DRAM bounce
                nm_d = DSCR.tile([ST], F16, tag=f"nm{s}")
                nc.sync.dma_start(out=nm_d[:], in_=negmu[:])
                ex_d = DSCR.tile([ST], F32, tag=f"ex{s}")
                nc.sync.dma_start(out=ex_d[:], in_=ex2_sb[:])
                mur = TMP.tile([P, 4], F16, tag="mur")
                nc.sync.dma_start(out=mur[:],
                                  in_=nm_d[:].rearrange("(g p) -> p g", p=P))
                ex2r = TMP.tile([P, 4], F32, tag="ex2r")
                nc.sync.dma_start(out=ex2r[:],
                                  in_=ex_d[:].rearrange("(g p) -> p g", p=P))

                # rstd via quake rsqrt (1 newton) - small [128,4] chain (DVE)
                nmu2 = TMP.tile([P, 4], F32, tag="nmu2")
                nc.vector.scalar_tensor_tensor(out=nmu2[:], in0=mur[:], scalar=-1.0,
                                               in1=mur[:], op0=ALU.mult, op1=ALU.mult)
                ve = TMP.tile([P, 4], F32, tag="ve")
                nc.vector.scalar_tensor_tensor(out=ve[:], in0=nmu2[:], scalar=1e-5,
                                               in1=ex2r[:], op0=ALU.add, op1=ALU.add)
                hsh = TMP.tile([P, 4], I32, tag="hsh")
                nc.vector.tensor_scalar(out=hsh[:], in0=ve[:].bitcast(I32), scalar1=1,
                                        scalar2=None, op0=ALU.logical_shift_right)
                nc.vector.tensor_scalar(out=hsh[:], in0=hsh[:], scalar1=-1,
                                        scalar2=MAGIC, op0=ALU.mult, op1=ALU.add)
                y0 = hsh[:].bitcast(F32)
                nt = TMP.tile([P, 4], F32, tag="nt")
                nc.vector.tensor_mul(out=nt[:], in0=y0, in1=y0)
                nc.vector.scalar_tensor_tensor(out=nt[:], in0=nt[:], scalar=-0.5,
                                               in1=ve[:], op0=ALU.mult, op1=ALU.mult)
                nc.vector.tensor_scalar_add(out=nt[:], in0=nt[:], scalar1=1.5)
                an_s = STR.tile([P, 4], F32, tag="an")
                nc.vector.tensor_mul(out=an_s[:], in0=y0, in1=nt[:])
                ascl_s = STR.tile([P, 4], F32, tag="ascl")
                nc.vector.tensor_scalar_mul(out=ascl_s[:], in0=an_s[:], scalar1=SCALE)
                abit_s = STR.tile([P, 4], F32, tag="abit")
                nc.vector.tensor_scalar_mul(out=abit_s[:], in0=ascl_s[:],
                                            scalar1=LOG2E8)
                ascls.append(ascl_s)
                abits.append(abit_s)
                if DEBUG:
                    nc.sync.dma_start(out=dbg["d_an"][s], in_=an_s[:])

                # shared kv lora: t2 = Avg @ xs_raw - mu*avg1
                t2p = CVKV.tile([R, ST], F32, tag="cv")
                nc.tensor.matmul(t2p[:], avT[:, 0, :], xsb_s[:, 0, :], start=True, stop=False)
                nc.tensor.matmul(t2p[:], avT[:, 1, :], xsb_s[:, 1, :], start=False, stop=False)
                nc.tensor.matmul(t2p[:], avg1, negmu[:], start=False, stop=True)
                t2 = TMP.tile([R, ST], F16, tag="t2")
                nc.vector.tensor_copy(out=t2[:], in_=t2p[:])

                # kv projections: kvoc 0,1 = K (permuted cols), 2,3 = V
                vtmp_s = STR.tile([P, 2, ST], F16, tag="vtmp")
                for kvoc in range(4):
                    wg1c = wg1k if kvoc < 2 else wg1v
                    bvc = bvTk if kvoc < 2 else bvTv
                    kps = CVKV.tile([P, ST], F32, tag="cv")
                    nc.tensor.matmul(kps[:], kvwT[:, 0, kvoc * P:(kvoc + 1) * P],
                                     xsb_s[:, 0, :], start=True, stop=False)
                    nc.tensor.matmul(kps[:], kvwT[:, 1, kvoc * P:(kvoc + 1) * P],
                                     xsb_s[:, 1, :], start=False, stop=False)
                    nc.tensor.matmul(kps[:], wg1c[:, kvoc % 2, :], negmu[:],
                                     start=False, stop=False)
                    nc.tensor.matmul(kps[:], bvc[:, kvoc % 2, :], t2[:],
                                     start=False, stop=True)
                    if kvoc < 2:
                        nc.vector.tensor_copy(
                            out=k8tmp[:, kvoc, s * ST:(s + 1) * ST], in_=kps[:])
                    else:
                        nc.vector.tensor_copy(out=vtmp_s[:, kvoc - 2, :], in_=kps[:])

                # v transpose (PE) + rstd scale -> vsb [m, ml, h, dh+1] fp16
                vsb_s = STR.tile([P, 4, HEAD, DH + 1], F16, tag="vsb")
                for vc in range(2):
                    for u4 in range(4):
                        vtp = CVKV.tile([P, P], F16, tag="cv")
                        nc.tensor.transpose(vtp[:],
                                            vtmp_s[:, vc, u4 * P:(u4 + 1) * P],
                                            ident)
                        nc.vector.tensor_scalar_mul(
                            out=vsb_s[:, u4, vc * 4:(vc + 1) * 4, 0:DH],
                            in0=vtp[:].rearrange("p (h d) -> p h d", d=DH),
                            scalar1=an_s[:, u4:u4 + 1])
                nc.gpsimd.memset(vsb_s[:, :, :, DH:DH + 1], 1.0)
                vsbs.append(vsb_s)
                if DEBUG:
                    nc.sync.dma_start(out=dbg["d_vsb"][s], in_=vsb_s[:])

                if s == 0:
                    # q path (rows 0:1024 of strip 0), permuted out channels
                    tqp = P1.tile([R, NB], F32, tag="p1")
                    for nh in range(2):
                        sl = slice(nh * 512, (nh + 1) * 512)
                        nc.tensor.matmul(tqp[:, sl], aqT[:, 0, :], xs_t[:, 0, sl],
                                         start=True, stop=False)
                        nc.tensor.matmul(tqp[:, sl], aqT[:, 1, :], xs_t[:, 1, sl],
                                         start=False, stop=True)
                    nc.vector.tensor_copy(out=tq[:], in_=tqp[:])
                    for oc in range(2):
                        qps = P1.tile([P, NB], F32, tag="p1")
                        for nh in range(2):
                            sl = slice(nh * 512, (nh + 1) * 512)
                            nc.tensor.matmul(qps[:, sl],
                                             qwT[:, 0, oc * P:(oc + 1) * P],
                                             xs_t[:, 0, sl], start=True, stop=False)
                            nc.tensor.matmul(qps[:, sl],
                                             qwT[:, 1, oc * P:(oc + 1) * P],
                                             xs_t[:, 1, sl], start=False, stop=False)
                            nc.tensor.matmul(qps[:, sl], bqT[:, oc, :], tq[:, sl],
                                             start=False, stop=True)
                        nc.vector.tensor_scalar_add(
                            out=q8tmp[:, oc, :], in0=qps[:], scalar1=wf[:, oc:oc + 1])
                    # repack q to 32-aligned head bases (one DMA per head)
                    for half, dst in ((0, qA), (1, qB)):
                        for a in range(4):
                            nc.sync.dma_start(
                                out=dst[32 * a:32 * a + 16],
                                in_=q8tmp[half * 64 + 16 * a:half * 64 + 16 * a + 16])

            # repack k (both strips in k8tmp), one DMA per head
            for half, dst in ((0, kA), (1, kB)):
                for a in range(4):
                    nc.sync.dma_start(
                        out=dst[32 * a:32 * a + 16],
                        in_=k8tmp[half * 64 + 16 * a:half * 64 + 16 * a + 16])
            if DEBUG:
                nc.sync.dma_start(out=dbg["d_k8"][:], in_=k8tmp[:].bitcast(U8))
                nc.sync.dma_start(out=dbg["d_q8"][:], in_=q8tmp[:].bitcast(U8))
                nc.sync.dma_start(out=dbg["d_kA"][:], in_=kA[:].bitcast(U8))
                nc.sync.dma_start(out=dbg["d_qA"][:], in_=qA[:].bitcast(U8))

            # ---------------- attention: per-head pipeline ----------------
            for h in range(HEAD):
                a = h % 4
                kT, qT8 = (kA, qA) if h < 4 else (kB, qB)
                ptsh = PTS.tile([P, 8, NB], F16, tag="ptsh")
                for mc in range(8):
                    s_, ml = mc // 4, mc % 4
                    stile = P1.tile([P, NB], F32, tag="p1")
                    for qc in range(4):
                        nc.tensor.matmul(
                            stile[:, qc * 256:(qc + 1) * 256],
                            kT[32 * a:32 * a + 16, :, mc * P:(mc + 1) * P],
                            qT8[32 * a:32 * a + 16, :, qc * 256:(qc + 1) * 256],
                            start=True, stop=True, perf_mode=PM.DoubleRow,
                            tile_position=(32 * a, 0))
                    if EXP_SCHED[mc] == "A":
                        nc.scalar.activation(out=ptsh[:, mc, :], in_=stile[:],
                                             func=AF.Exp,
                                             scale=ascls[s_][:, ml:ml + 1])
                    else:
                        nc.vector.tensor_scalar(
                            out=ptsh[:, mc, :].bitcast(I16), in0=stile[:],
                            scalar1=abits[s_][:, ml:ml + 1], scalar2=BITB,
                            op0=ALU.mult, op1=ALU.add)

                pvq = PV.tile([P, 8, DH + 1], F32, tag="pv")
                for qb in range(8):
                    for mc in range(8):
                        nc.tensor.matmul(
                            pvq[:, qb, :],
                            ptsh[:, mc, qb * P:(qb + 1) * P],
                            vsbs[mc // 4][:, mc % 4, h, :],
                            start=(mc == 0), stop=(mc == 7),
                            tile_position=(0, 0))
                rec = TMP.tile([P, 8], F32, tag="rec")
                nc.vector.reciprocal(out=rec[:], in_=pvq[:, :, DH])
                nc.vector.tensor_tensor(out=onorm[:, :, h, :],
                                        in0=pvq[:, :, 0:DH],
                                        in1=s0(rec[:], DH), op=ALU.mult)
                if DEBUG:
                    if h == 0:
                        nc.sync.dma_start(out=dbg["d_pts0"][:], in_=ptsh[:])
                    nc.sync.dma_start(out=dbg["d_rec"][h], in_=rec[:])

            # ---------------- transpose + projection ----------------
            for qb in range(8):
                for ch in range(2):
                    ttile = CVKV.tile([P, P], F16, tag="cv")
                    nc.tensor.transpose(
                        ttile[:],
                        onorm[:, qb, ch * 4:(ch + 1) * 4, :].rearrange(
                            "p h d -> p (h d)"),
                        ident)
                    nc.vector.tensor_copy(
                        out=outT[:, ch, qb * P:(qb + 1) * P], in_=ttile[:])

            if DEBUG:
                nc.sync.dma_start(out=dbg["d_onorm"][:], in_=onorm[:])
                nc.sync.dma_start(out=dbg["d_outT"][:], in_=outT[:])
            for i4 in range(4):
                pp = P1.tile([P, 2, C], F32, tag="p1")
                fin = FIN.tile([P, 2, C], F32, tag="fin")
                for tt in range(2):
                    t8 = 2 * i4 + tt
                    nc.tensor.matmul(pp[:, tt, :], outT[:, 0, t8 * P:(t8 + 1) * P],
                                     pwT[:, 0, :], start=True, stop=False)
                    nc.tensor.matmul(pp[:, tt, :], outT[:, 1, t8 * P:(t8 + 1) * P],
                                     pwT[:, 1, :], start=False, stop=True)
                    nc.vector.tensor_add(out=fin[:, tt, :], in0=pp[:, tt, :],
                                         in1=pbB[:])
                nc.sync.dma_start(
                    out=out_d[i4 * 256:(i4 + 1) * 256].rearrange(
                        "(t p) c -> p t c", t=2),
                    in_=fin[:])

    nc.finalize()
    return nc


def _kq_perm():
    # PSUM chunk oc holds half `oc` of all heads: partition p=16h+lane ->
    # original channel 32h + 16*oc + lane
    perm = np.empty(2 * P_, np.int64)
    for oc in range(2):
        for p in range(P_):
            perm[oc * P_ + p] = 32 * (p // 16) + 16 * oc + (p % 16)
    return perm


P_ = 128


def _prep_shared(q_w, q_b, kv_w, kv_b, proj_w, proj_b, a_q, b_q, a_v, b_v,
                 sr_w, sr_b, ln_g, ln_b):
    f32 = np.float32

    def chunkT(w):  # [out, in] -> [128, n_in_chunks, out]
        wt = np.ascontiguousarray(np.asarray(w, f32).T)
        ic, oc = wt.shape
        return np.ascontiguousarray(
            wt.reshape(ic // 128, 128, oc).transpose(1, 0, 2)).astype(F16NP)

    kv_w = np.asarray(kv_w, f32)
    a_v = np.asarray(a_v, f32)
    b_v = np.asarray(b_v, f32)
    b_q = np.asarray(b_q, f32)
    q_w = np.asarray(q_w, f32)
    q_b = np.asarray(q_b, f32)
    g = np.asarray(ln_g, f32)
    bb = np.asarray(ln_b, f32)
    proj_w = np.asarray(proj_w, f32)

    # fold LayerNorm gamma into kv/a_v weights; mean via rank-1 correction;
    # k-side constants dropped (softmax shift invariance), v-side constants
    # folded into the projection bias.
    Wg = kv_w * g[None, :]
    wg1 = Wg.sum(1)
    Avg = a_v * g[None, :]
    avg1 = Avg.sum(1)
    wbt = kv_w @ bb + np.asarray(kv_b, f32)
    dconst = b_v @ (a_v @ bb)
    wv_const = wbt[C:] + dconst
    pb_eff = np.asarray(proj_b, f32) + proj_w @ wv_const

    perm = _kq_perm()
    qw_p = q_w[perm]
    qb_p = q_b[perm]
    bq_p = b_q[perm]
    Wgk_p = Wg[0:C][perm]
    wg1k_p = wg1[0:C][perm]
    bvk_p = b_v[perm]

    qwT = chunkT(qw_p)                                 # [128, 2, 256]
    kvwT = chunkT(np.concatenate([Wgk_p, Wg[C:]], 0))  # [128, 2, 512]
    pwT = chunkT(proj_w)
    srwT = np.asarray(sr_w, f32).transpose(1, 2, 3, 0).reshape(2, 128, 4, C)
    srwT = np.ascontiguousarray(srwT.transpose(1, 0, 2, 3)).astype(F16NP)
    aqT = chunkT(a_q)                                  # [128, 2, 8]
    avT = chunkT(Avg)
    bqT = np.ascontiguousarray(bq_p.T.reshape(R, 2, 128)).astype(F16NP)
    bvTk = np.ascontiguousarray(bvk_p.T.reshape(R, 2, 128)).astype(F16NP)
    bvTv = np.ascontiguousarray(b_v.T.reshape(R, 2, 128)).astype(F16NP)

    blob = np.zeros((128, BLOB_COLS), F16NP)
    blob[:, O_QWT:O_QWT + 512] = qwT.reshape(128, 512)
    blob[:, O_KVWT:O_KVWT + 1024] = kvwT.reshape(128, 1024)
    blob[:, O_PWT:O_PWT + 512] = pwT.reshape(128, 512)
    blob[:, O_SRWT:O_SRWT + 2048] = srwT.reshape(128, 2048)
    blob[:, O_AQT:O_AQT + 16] = aqT.reshape(128, 16)
    blob[:, O_AVT:O_AVT + 16] = avT.reshape(128, 16)
    blob[:, O_IDENT:O_IDENT + 128] = np.eye(128, dtype=F16NP)
    blob[0:R, O_BQT:O_BQT + 256] = bqT.reshape(R, 256)
    blob[0:R, O_BVTK:O_BVTK + 256] = bvTk.reshape(R, 256)
    blob[0:R, O_BVTV:O_BVTV + 256] = bvTv.reshape(R, 256)
    blob[0, O_WG1K:O_WG1K + 256] = wg1k_p.reshape(2, 128).astype(F16NP).reshape(256)
    blob[0, O_WG1V:O_WG1V + 256] = wg1[C:].reshape(2, 128).astype(F16NP).reshape(256)
    blob[0, O_AVG1:O_AVG1 + 8] = avg1.astype(F16NP)
    blob[:, O_ONE1C] = F16NP(1.0 / C)

    def pcols(v):  # [n*128] -> [128, n]
        v = np.asarray(v, f32)
        return np.ascontiguousarray(v.reshape(-1, 128).T)

    wf = np.zeros((128, 4), f32)
    wf[:, 0:2] = pcols(qb_p)
    wf[:, 2:4] = pcols(np.asarray(sr_b, f32))
    return dict(wb=blob, wf=wf, pb=pb_eff)


def kernel(x, q_w, q_b, kv_w, kv_b, proj_w, proj_b, a_q, b_q, a_v, b_v,
           sr_w, sr_b, ln_g, ln_b, H, W):
    from concourse.bass_utils import run_bass_kernel_spmd

    x = np.asarray(x, np.float32)
    assert x.shape == (B, N, C) and int(H) == 64 and int(W) == 64

    if "nc" not in _CACHE:
        _CACHE["nc"] = _build_program()
    nc = _CACHE["nc"]

    shared = _prep_shared(q_w, q_b, kv_w, kv_b, proj_w, proj_b, a_q, b_q,
                          a_v, b_v, sr_w, sr_b, ln_g, ln_b)
    in_maps = []
    for c in range(NCORES):
        b, j = c // 4, c % 4
        xb = np.roll(x[b], -NB * j, axis=0)             # own block at rows 0:1024
        xT = np.ascontiguousarray(xb.T.astype(F16NP))   # [256, 4096]
        xT = np.ascontiguousarray(
            xT.reshape(2, 128, N).transpose(1, 0, 2))   # [128, 2, 4096]
        in_maps.append(dict(shared, xT=xT))

    res = run_bass_kernel_spmd(nc, in_maps, list(range(NCORES)))
    _CACHE["res"] = res
    out = np.empty((B, N, C), np.float32)
    for c in range(NCORES):
        b, j = c // 4, c % 4
        out[b, NB * j:NB * (j + 1)] = res.results[c]["out"]
    return out


# revision 18
# speedup vs baseline: 1.3982x; 1.1300x over previous
"""Trainium2 Bass kernel for PVT-style spatial-reduction attention (v3).

Shapes (hardcoded): x [2, 4096, 256], HEAD=8, dh=32, SR=2, R=8, H=W=64.
Sharding: core c = (batch b = c//4, query block j = c%4). Each core computes
q/attention/proj for its 1024 query rows and redundantly computes the small
conv+LN+KV path for its batch. Per-core x is pre-rotated on host so each
core's own query block is rows 0:1024.

v3 design:
- fp16 compute chain (conv/kv/q/pv/proj); fp8e4m3 DoubleRow score matmuls
  (0.5 cyc/row) via host-permuted q/k projection columns + per-head repack
  DMAs to 32-aligned lane bases.
- Transposed pv (pts stationary): 33-row pv matmuls, per-partition softmax
  denominators (single reciprocal + stride-0 scale, no broadcasts).
- V computed token-major directly (xsb stationary) - no PE transposes.
- Softmax exp split across ScalarE (true Exp) and DVE (one-op Schraudolph
  int16 bit-trick into fp16); per-head schedule interleaves the engines.
- Attention starts after strip 0: scores for kv blocks 0-3 of all heads are
  emitted interleaved with strip-1 conv/kv so ScalarE/DVE start ~15us in.
- Few large DMAs; conv weights in a separate first blob for an early start.

PSUM (8 banks): P1 2x[128,1024]f32 (4) for q-path/scores/proj, CVKV
2x[128,512]f32 (2) for conv/kv/v/transposes, PV 2x[128,8,33]f32 (2) for pv.
"""
import sys

if "/opt/trn_rl_repo" not in sys.path:
    sys.path.insert(0, "/opt/trn_rl_repo")

import numpy as np

F16NP = np.float16

HEAD, DH, C, N, B, M, R = 8, 32, 256, 4096, 2, 1024, 8
NB = N // 4          # query rows per core
SCALE = DH ** -0.5
NCORES = 8
MAGIC = 0x5F3759DF
LOG2E8 = 1477.3195879  # 2^10 / ln 2
BITB = 15317.95        # tuned fp16 Schraudolph offset (trunc semantics)

# per-head exp engine schedule over mc=0..7 (A=ScalarE exp, D=DVE bit trick);
# alternating 6A2D / 5A3D heads -> 44 ACT / 20 DVE tiles.
EXP_SCHED = ["AADAADAA", "ADAADAAD"] * 4

# conv blob (wbc): srwT 2048 | ones1c 1
OC_SRWT, OC_ONE1C = 0, 2048
WBC_COLS = 2064
# main blob (wb) column offsets (fp16 elements)
O_QWT, O_KVWT, O_PWT = 0, 512, 1536
O_AQT, O_AVT, O_IDENT, O_BQT = 2048, 2064, 2080, 2208
O_BVTK, O_BVTV, O_WG1K, O_WG1V = 2464, 2720, 2976, 3232
O_AVG1 = 3488
BLOB_COLS = 3504

_CACHE = {}
DEBUG = False


def _build_program():
    import concourse.bass as bass
    import concourse.tile as tile
    from concourse.bacc import Bacc
    from concourse import mybir

    F32 = mybir.dt.float32
    F16 = mybir.dt.float16
    FP8 = mybir.dt.float8e4
    I16 = mybir.dt.int16
    I32 = mybir.dt.int32
    U8 = mybir.dt.uint8
    AF = mybir.ActivationFunctionType
    ALU = mybir.AluOpType
    PM = mybir.MatmulPerfMode

    nc = Bacc()
    P = 128
    ST = 512       # kv tokens per strip

    def s0(ap, n):
        # stride-0 broadcast along a new innermost free dim
        return bass.AP(tensor=ap.tensor, offset=ap.offset,
                       ap=[list(d) for d in ap.ap] + [[0, n]])

    def bcast(ap, nparts):
        return bass.AP(tensor=ap.tensor, offset=ap.offset,
                       ap=[[0, nparts]] + [list(d) for d in ap.ap])

    xT_d = nc.declare_dram_parameter("xT", [P, 2, N], F16, isOutput=False)
    wbc_d = nc.declare_dram_parameter("wbc", [P, WBC_COLS], F16, isOutput=False)
    wb_d = nc.declare_dram_parameter("wb", [P, BLOB_COLS], F16, isOutput=False)
    wf_d = nc.declare_dram_parameter("wf", [P, 4], F32, isOutput=False)
    pb_d = nc.declare_dram_parameter("pb", [C], F32, isOutput=False)
    out_d = nc.declare_dram_parameter("out", [NB, C], F32, isOutput=True)
    if DEBUG:
        dbg = {
            "d_xsb": nc.declare_dram_parameter("d_xsb", [2, P, 2, 512], F16, isOutput=True),
            "d_an": nc.declare_dram_parameter("d_an", [2, P, 4], F32, isOutput=True),
            "d_k8": nc.declare_dram_parameter("d_k8", [P, 2, M], U8, isOutput=True),
            "d_q8": nc.declare_dram_parameter("d_q8", [P, 2, NB], U8, isOutput=True),
            "d_kA": nc.declare_dram_parameter("d_kA", [P, 2, M], U8, isOutput=True),
            "d_qA": nc.declare_dram_parameter("d_qA", [P, 2, NB], U8, isOutput=True),
            "d_vsb": nc.declare_dram_parameter("d_vsb", [2, P, 4, HEAD, DH + 1], F16, isOutput=True),
            "d_pts0": nc.declare_dram_parameter("d_pts0", [P, 8, NB], F16, isOutput=True),
            "d_rec": nc.declare_dram_parameter("d_rec", [HEAD, P, 8], F32, isOutput=True),
            "d_onorm": nc.declare_dram_parameter("d_onorm", [P, 8, HEAD, DH], F16, isOutput=True),
            "d_outT": nc.declare_dram_parameter("d_outT", [P, 2, NB], F16, isOutput=True),
        }

    with tile.TileContext(nc) as tc:
        with tc.tile_pool(name="wgt", bufs=1) as WGT, \
             tc.tile_pool(name="acts", bufs=1) as ACTS, \
             tc.tile_pool(name="str", bufs=2) as STR, \
             tc.tile_pool(name="tmp", bufs=2) as TMP, \
             tc.tile_pool(name="pts", bufs=8) as PTS, \
             tc.tile_pool(name="fin", bufs=2) as FIN, \
             tc.tile_pool(name="p1", bufs=2, space="PSUM") as P1, \
             tc.tile_pool(name="cvkv", bufs=2, space="PSUM") as CVKV, \
             tc.tile_pool(name="pv", bufs=2, space="PSUM") as PV, \
             tc.tile_pool(name="dscr", bufs=2, space="DRAM") as DSCR:

            # ---------------- loads (order matters for early start) ----------
            wbc = WGT.tile([P, WBC_COLS], F16, tag="wbc")
            nc.sync.dma_start(out=wbc[:], in_=wbc_d[:])
            xs0 = ACTS.tile([P, 2, 2048], F16, tag="xT0")
            nc.sync.dma_start(out=xs0[:], in_=xT_d[:, :, 0:2048])
            wb = WGT.tile([P, BLOB_COLS], F16, tag="wb")
            nc.sync.dma_start(out=wb[:], in_=wb_d[:])
            xs1 = ACTS.tile([P, 2, 2048], F16, tag="xT1")
            nc.sync.dma_start(out=xs1[:], in_=xT_d[:, :, 2048:4096])
            wf = WGT.tile([P, 4], F32, tag="wf")
            nc.sync.dma_start(out=wf[:], in_=wf_d[:])
            pbB = WGT.tile([P, C], F32, tag="pbB")
            nc.sync.dma_start(out=pbB[:], in_=bcast(pb_d.ap(), P))
            xTs = [xs0, xs1]

            srwT = wbc[:, OC_SRWT:OC_SRWT + 2048].rearrange(
                "p (ch t c) -> p ch t c", ch=2, t=4)
            ones1c = wbc[:, OC_ONE1C:OC_ONE1C + 1]
            qwT = wb[:, O_QWT:O_QWT + 512].rearrange("p (ch c) -> p ch c", ch=2)
            kvwT = wb[:, O_KVWT:O_KVWT + 1024].rearrange("p (ch c) -> p ch c", ch=2)
            pwT = wb[:, O_PWT:O_PWT + 512].rearrange("p (ch c) -> p ch c", ch=2)
            aqT = wb[:, O_AQT:O_AQT + 16].rearrange("p (ch r) -> p ch r", ch=2)
            avT = wb[:, O_AVT:O_AVT + 16].rearrange("p (ch r) -> p ch r", ch=2)
            ident = wb[:, O_IDENT:O_IDENT + 128]
            bqT = wb[0:R, O_BQT:O_BQT + 256].rearrange("r (ch c) -> r ch c", ch=2)
            bvTk = wb[0:R, O_BVTK:O_BVTK + 256].rearrange("r (ch c) -> r ch c", ch=2)
            bvTv = wb[0:R, O_BVTV:O_BVTV + 256]
            wg1k = wb[0:1, O_WG1K:O_WG1K + 256].rearrange("a (ch c) -> a ch c", ch=2)
            wg1v = wb[0:1, O_WG1V:O_WG1V + 256]
            avg1 = wb[0:1, O_AVG1:O_AVG1 + 8]

            # persistent activations
            k8tmp = ACTS.tile([P, 2, M], FP8, tag="k8tmp")
            q8tmp = ACTS.tile([P, 2, NB], FP8, tag="q8tmp")
            kA = ACTS.tile([P, 2, M], FP8, tag="kA")
            kB = ACTS.tile([P, 2, M], FP8, tag="kB")
            qA = ACTS.tile([P, 2, NB], FP8, tag="qA")
            qB = ACTS.tile([P, 2, NB], FP8, tag="qB")
            outT = ACTS.tile([P, 2, NB], F16, tag="outT")
            onorm = ACTS.tile([P, 8, HEAD, DH], F16, tag="onorm")
            tq = ACTS.tile([R, NB], F16, tag="tq")

            xsbs, sqs, negmus, t2s, vsbs, ans, ascls, abits = \
                [], [], [], [], [], [], [], []

            # ---------------- per-strip pieces as emission chunks ------------
            def conv_oc(s, oc):
                xs_t = xTs[s]
                if oc == 0:
                    xsb_s = STR.tile([P, 2, ST], F16, tag="xsb")
                    xsbs.append(xsb_s)
                xsb_s = xsbs[s]
                cps = CVKV.tile([P, ST], F32, tag="cv")
                first = True
                for cc in range(2):
                    xv = xs_t[:, cc, :].rearrange(
                        "p (i a j b) -> p i a j b", i=16, a=2, j=32, b=2)
                    for di in range(2):
                        for dj in range(2):
                            nc.tensor.matmul(
                                cps[:], srwT[:, cc, di * 2 + dj,
                                             oc * P:(oc + 1) * P],
                                xv[:, :, di, :, dj],
                                start=first,
                                stop=(cc == 1 and di == 1 and dj == 1))
                            first = False
                nc.vector.tensor_scalar_add(
                    out=xsb_s[:, oc, :], in0=cps[:], scalar1=wf[:, 2 + oc:3 + oc])
                if DEBUG and oc == 1:
                    nc.sync.dma_start(out=dbg["d_xsb"][s], in_=xsb_s[:])

            def stats(s):
                xsb_s = xsbs[s]
                sq_s = STR.tile([P, 2, ST], F16, tag="sq")
                nc.gpsimd.tensor_mul(out=sq_s[:], in0=xsb_s[:], in1=xsb_s[:])
                sxp = CVKV.tile([1, ST], F32, tag="cv")
                nc.tensor.matmul(sxp[:], ones1c, xsb_s[:, 0, :], start=True, stop=False)
                nc.tensor.matmul(sxp[:], ones1c, xsb_s[:, 1, :], start=False, stop=True)
                negmu = TMP.tile([1, ST], F16, tag="negmu")
                nc.vector.tensor_scalar_mul(out=negmu[:], in0=sxp[:], scalar1=-1.0)
                negmus.append(negmu)
                sxxp = CVKV.tile([1, ST], F32, tag="cv")
                nc.tensor.matmul(sxxp[:], ones1c, sq_s[:, 0, :], start=True, stop=False)
                nc.tensor.matmul(sxxp[:], ones1c, sq_s[:, 1, :], start=False, stop=True)
                ex2_sb = TMP.tile([1, ST], F32, tag="ex2sb")
                nc.vector.tensor_copy(out=ex2_sb[:], in_=sxxp[:])

                # chunk-major repack [1,512] -> [128,4] via DRAM bounce
                nm_d = DSCR.tile([ST], F16, tag=f"nm{s}")
                nc.sync.dma_start(out=nm_d[:], in_=negmu[:])
                ex_d = DSCR.tile([ST], F32, tag=f"ex{s}")
                nc.sync.dma_start(out=ex_d[:], in_=ex2_sb[:])
                mur = TMP.tile([P, 4], F16, tag="mur")
                nc.sync.dma_start(out=mur[:],
                                  in_=nm_d[:].rearrange("(g p) -> p g", p=P))
                ex2r = TMP.tile([P, 4], F32, tag="ex2r")
                nc.sync.dma_start(out=ex2r[:],
                                  in_=ex_d[:].rearrange("(g p) -> p g", p=P))

                # rstd via quake rsqrt (1 newton), [128,4] chain on DVE
                nmu2 = TMP.tile([P, 4], F32, tag="nmu2")
                nc.vector.scalar_tensor_tensor(out=nmu2[:], in0=mur[:], scalar=-1.0,
                                               in1=mur[:], op0=ALU.mult, op1=ALU.mult)
                ve = TMP.tile([P, 4], F32, tag="ve")
                nc.vector.scalar_tensor_tensor(out=ve[:], in0=nmu2[:], scalar=1e-5,
                                               in1=ex2r[:], op0=ALU.add, op1=ALU.add)
                hsh = TMP.tile([P, 4], I32, tag="hsh")
                nc.vector.tensor_scalar(out=hsh[:], in0=ve[:].bitcast(I32), scalar1=1,
                                        scalar2=None, op0=ALU.logical_shift_right)
                nc.vector.tensor_scalar(out=hsh[:], in0=hsh[:], scalar1=-1,
                                        scalar2=MAGIC, op0=ALU.mult, op1=ALU.add)
                y0 = hsh[:].bitcast(F32)
                nt = TMP.tile([P, 4], F32, tag="nt")
                nc.vector.tensor_mul(out=nt[:], in0=y0, in1=y0)
                nc.vector.scalar_tensor_tensor(out=nt[:], in0=nt[:], scalar=-0.5,
                                               in1=ve[:], op0=ALU.mult, op1=ALU.mult)
                nc.vector.tensor_scalar_add(out=nt[:], in0=nt[:], scalar1=1.5)
                an_s = STR.tile([P, 4], F32, tag="an")
                nc.vector.tensor_mul(out=an_s[:], in0=y0, in1=nt[:])
                ascl_s = STR.tile([P, 4], F32, tag="ascl")
                nc.vector.tensor_scalar_mul(out=ascl_s[:], in0=an_s[:], scalar1=SCALE)
                abit_s = STR.tile([P, 4], F32, tag="abit")
                nc.vector.tensor_scalar_mul(out=abit_s[:], in0=ascl_s[:],
                                            scalar1=LOG2E8)
                ans.append(an_s)
                ascls.append(ascl_s)
                abits.append(abit_s)
                if DEBUG:
                    nc.sync.dma_start(out=dbg["d_an"][s], in_=an_s[:])

            def t2_and_k(s):
                xsb_s, negmu = xsbs[s], negmus[s]
                t2p = CVKV.tile([R, ST], F32, tag="cv")
                nc.tensor.matmul(t2p[:], avT[:, 0, :], xsb_s[:, 0, :], start=True, stop=False)
                nc.tensor.matmul(t2p[:], avT[:, 1, :], xsb_s[:, 1, :], start=False, stop=False)
                nc.tensor.matmul(t2p[:], avg1, negmu[:], start=False, stop=True)
                t2 = TMP.tile([R, ST], F16, tag="t2")
                nc.vector.tensor_copy(out=t2[:], in_=t2p[:])
                t2s.append(t2)
                for kvoc in range(2):
                    kps = CVKV.tile([P, ST], F32, tag="cv")
                    nc.tensor.matmul(kps[:], kvwT[:, 0, kvoc * P:(kvoc + 1) * P],
                                     xsb_s[:, 0, :], start=True, stop=False)
                    nc.tensor.matmul(kps[:], kvwT[:, 1, kvoc * P:(kvoc + 1) * P],
                                     xsb_s[:, 1, :], start=False, stop=False)
                    nc.tensor.matmul(kps[:], wg1k[:, kvoc, :], negmu[:],
                                     start=False, stop=False)
                    nc.tensor.matmul(kps[:], bvTk[:, kvoc, :], t2[:],
                                     start=False, stop=True)
                    nc.vector.tensor_copy(
                        out=k8tmp[:, kvoc, s * ST:(s + 1) * ST], in_=kps[:])

            def k_repack(s):
                for half, dst in ((0, kA), (1, kB)):
                    for a in range(4):
                        nc.sync.dma_start(
                            out=dst[32 * a:32 * a + 16, :, s * ST:(s + 1) * ST],
                            in_=k8tmp[half * 64 + 16 * a:half * 64 + 16 * a + 16,
                                      :, s * ST:(s + 1) * ST])

            def v_path(s):
                # token-major v: out[m, vchan] via xsb/negmu/t2 as stationary
                xsb_s, negmu, t2, an_s = xsbs[s], negmus[s], t2s[s], ans[s]
                vsb_s = STR.tile([P, 4, HEAD, DH + 1], F16, tag="vsb")
                for ml in range(4):
                    sl = slice(ml * P, (ml + 1) * P)
                    vP = CVKV.tile([P, C], F32, tag="cv")
                    nc.tensor.matmul(vP[:], xsb_s[:, 0, sl], kvwT[:, 0, 256:512],
                                     start=True, stop=False)
                    nc.tensor.matmul(vP[:], xsb_s[:, 1, sl], kvwT[:, 1, 256:512],
                                     start=False, stop=False)
                    nc.tensor.matmul(vP[:], negmu[:, sl], wg1v,
                                     start=False, stop=False, tile_position=(0, 0))
                    nc.tensor.matmul(vP[:], t2[:, sl], bvTv,
                                     start=False, stop=True, tile_position=(0, 0))
                    nc.vector.tensor_scalar_mul(
                        out=vsb_s[:, ml, :, 0:DH],
                        in0=vP[:].rearrange("p (h d) -> p h d", d=DH),
                        scalar1=an_s[:, ml:ml + 1])
                nc.gpsimd.memset(vsb_s[:, :, :, DH:DH + 1], 1.0)
                vsbs.append(vsb_s)
                if DEBUG:
                    nc.sync.dma_start(out=dbg["d_vsb"][s], in_=vsb_s[:])

            def q_path():
                xs_t = xTs[0]
                tqp = P1.tile([R, NB], F32, tag="p1")
                for nh in range(2):
                    sl = slice(nh * 512, (nh + 1) * 512)
                    nc.tensor.matmul(tqp[:, sl], aqT[:, 0, :], xs_t[:, 0, sl],
                                     start=True, stop=False)
                    nc.tensor.matmul(tqp[:, sl], aqT[:, 1, :], xs_t[:, 1, sl],
                                     start=False, stop=True)
                nc.vector.tensor_copy(out=tq[:], in_=tqp[:])
                for oc in range(2):
                    qps = P1.tile([P, NB], F32, tag="p1")
                    for nh in range(2):
                        sl = slice(nh * 512, (nh + 1) * 512)
                        nc.tensor.matmul(qps[:, sl],
                                         qwT[:, 0, oc * P:(oc + 1) * P],
                                         xs_t[:, 0, sl], start=True, stop=False)
                        nc.tensor.matmul(qps[:, sl],
                                         qwT[:, 1, oc * P:(oc + 1) * P],
                                         xs_t[:, 1, sl], start=False, stop=False)
                        nc.tensor.matmul(qps[:, sl], bqT[:, oc, :], tq[:, sl],
                                         start=False, stop=True)
                    nc.vector.tensor_scalar_add(
                        out=q8tmp[:, oc, :], in0=qps[:], scalar1=wf[:, oc:oc + 1])
                for half, dst in ((0, qA), (1, qB)):
                    for a in range(4):
                        nc.sync.dma_start(
                            out=dst[32 * a:32 * a + 16],
                            in_=q8tmp[half * 64 + 16 * a:half * 64 + 16 * a + 16])

            # ---------------- attention pieces ----------------
            ptshs = [None] * HEAD

            def score_pair(h, mcp):
                # scores+exp for mc = 2*mcp, 2*mcp+1
                a = h % 4
                kT, qT8 = (kA, qA) if h < 4 else (kB, qB)
                if ptshs[h] is None:
                    ptsh = PTS.tile([P, 8, NB], F16, tag="ptsh")
                    ptshs[h] = ptsh
                ptsh = ptshs[h]
                for mc in (2 * mcp, 2 * mcp + 1):
                    s_, ml = mc // 4, mc % 4
                    stile = P1.tile([P, NB], F32, tag="p1")
                    for qc in range(4):
                        nc.tensor.matmul(
                            stile[:, qc * 256:(qc + 1) * 256],
                            kT[32 * a:32 * a + 16, :, mc * P:(mc + 1) * P],
                            qT8[32 * a:32 * a + 16, :, qc * 256:(qc + 1) * 256],
                            start=True, stop=True, perf_mode=PM.DoubleRow,
                            tile_position=(32 * a, 0))
                    if EXP_SCHED[h][mc] == "A":
                        nc.scalar.activation(out=ptsh[:, mc, :], in_=stile[:],
                                             func=AF.Exp,
                                             scale=ascls[s_][:, ml:ml + 1])
                    else:
                        nc.vector.tensor_scalar(
                            out=ptsh[:, mc, :].bitcast(I16), in0=stile[:],
                            scalar1=abits[s_][:, ml:ml + 1], scalar2=BITB,
                            op0=ALU.mult, op1=ALU.add)

            def pv_head(h):
                ptsh = ptshs[h]
                pvq = PV.tile([P, 8, DH + 1], F32, tag="pv")
                for qb in range(8):
                    for mc in range(8):
                        nc.tensor.matmul(
                            pvq[:, qb, :],
                            ptsh[:, mc, qb * P:(qb + 1) * P],
                            vsbs[mc // 4][:, mc % 4, h, :],
                            start=(mc == 0), stop=(mc == 7),
                            tile_position=(0, 0))
                rec = TMP.tile([P, 8], F32, tag="rec")
                nc.vector.reciprocal(out=rec[:], in_=pvq[:, :, DH])
                nc.vector.tensor_tensor(out=onorm[:, :, h, :],
                                        in0=pvq[:, :, 0:DH],
                                        in1=s0(rec[:], DH), op=ALU.mult)
                if DEBUG:
                    if h == 0:
                        nc.sync.dma_start(out=dbg["d_pts0"][:], in_=ptsh[:])
                    nc.sync.dma_start(out=dbg["d_rec"][h], in_=rec[:])

            def transpose_ch(ch):
                for qb in range(8):
                    ttile = CVKV.tile([P, P], F16, tag="cv")
                    nc.tensor.transpose(
                        ttile[:],
                        onorm[:, qb, ch * 4:(ch + 1) * 4, :].rearrange(
                            "p h d -> p (h d)"),
                        ident)
                    nc.vector.tensor_copy(
                        out=outT[:, ch, qb * P:(qb + 1) * P], in_=ttile[:])

            # ---------------- emission schedule ----------------
            conv_oc(0, 0)
            conv_oc(0, 1)
            stats(0)
            t2_and_k(0)
            k_repack(0)
            q_path()
            v_path(0)

            # interleave strip 1 with early scores (all heads, kv blocks 0-3)
            s1_chunks = [
                lambda: conv_oc(1, 0),
                lambda: conv_oc(1, 1),
                lambda: stats(1),
                lambda: t2_and_k(1),
                lambda: (k_repack(1), v_path(1)),
            ]
            b1_chunks = [(h, mcp) for h in range(HEAD) for mcp in (0, 1)]
            bi = 0
            for chunk in s1_chunks:
                chunk()
                for _ in range(2):
                    if bi < len(b1_chunks):
                        score_pair(*b1_chunks[bi])
                        bi += 1
            while bi < len(b1_chunks):
                score_pair(*b1_chunks[bi])
                bi += 1

            # second half: kv blocks 4-7 per head, pv pipelined one head behind
            for h in range(HEAD):
                score_pair(h, 2)
                score_pair(h, 3)
                if h > 0:
                    pv_head(h - 1)
                if h == 4:
                    transpose_ch(0)   # heads 0-3 normalized by now
            pv_head(HEAD - 1)
            if DEBUG:
                nc.sync.dma_start(out=dbg["d_onorm"][:], in_=onorm[:])
            transpose_ch(1)
            if DEBUG:
                nc.sync.dma_start(out=dbg["d_outT"][:], in_=outT[:])

            # ---------------- projection ----------------
            for i4 in range(4):
                pp = P1.tile([P, 2, C], F32, tag="p1")
                fin = FIN.tile([P, 2, C], F32, tag="fin")
                for tt in range(2):
                    t8 = 2 * i4 + tt
                    nc.tensor.matmul(pp[:, tt, :], outT[:, 0, t8 * P:(t8 + 1) * P],
                                     pwT[:, 0, :], start=True, stop=False)
                    nc.tensor.matmul(pp[:, tt, :], outT[:, 1, t8 * P:(t8 + 1) * P],
                                     pwT[:, 1, :], start=False, stop=True)
                    nc.vector.tensor_add(out=fin[:, tt, :], in0=pp[:, tt, :],
                                         in1=pbB[:])
                nc.sync.dma_start(
                    out=out_d[i4 * 256:(i4 + 1) * 256].rearrange(
                        "(t p) c -> p t c", t=2),
                    in_=fin[:])

    nc.finalize()
    return nc


P_ = 128


def _kq_perm():
    # PSUM chunk oc holds half `oc` of all heads: partition p=16h+lane ->
    # original channel 32h + 16*oc + lane
    perm = np.empty(2 * P_, np.int64)
    for oc in range(2):
        for p in range(P_):
            perm[oc * P_ + p] = 32 * (p // 16) + 16 * oc + (p % 16)
    return perm


def _prep_shared(q_w, q_b, kv_w, kv_b, proj_w, proj_b, a_q, b_q, a_v, b_v,
                 sr_w, sr_b, ln_g, ln_b):
    f32 = np.float32

    def chunkT(w):  # [out, in] -> [128, n_in_chunks, out]
        wt = np.ascontiguousarray(np.asarray(w, f32).T)
        ic, oc = wt.shape
        return np.ascontiguousarray(
            wt.reshape(ic // 128, 128, oc).transpose(1, 0, 2)).astype(F16NP)

    kv_w = np.asarray(kv_w, f32)
    a_v = np.asarray(a_v, f32)
    b_v = np.asarray(b_v, f32)
    b_q = np.asarray(b_q, f32)
    q_w = np.asarray(q_w, f32)
    q_b = np.asarray(q_b, f32)
    g = np.asarray(ln_g, f32)
    bb = np.asarray(ln_b, f32)
    proj_w = np.asarray(proj_w, f32)

    # fold LayerNorm gamma into kv/a_v weights; mean via rank-1 correction;
    # k-side constants dropped (softmax shift invariance), v-side constants
    # folded into the projection bias.
    Wg = kv_w * g[None, :]
    wg1 = Wg.sum(1)
    Avg = a_v * g[None, :]
    avg1 = Avg.sum(1)
    wbt = kv_w @ bb + np.asarray(kv_b, f32)
    dconst = b_v @ (a_v @ bb)
    wv_const = wbt[C:] + dconst
    pb_eff = np.asarray(proj_b, f32) + proj_w @ wv_const

    perm = _kq_perm()
    qw_p = q_w[perm]
    qb_p = q_b[perm]
    bq_p = b_q[perm]
    Wgk_p = Wg[0:C][perm]
    wg1k_p = wg1[0:C][perm]
    bvk_p = b_v[perm]

    qwT = chunkT(qw_p)                                 # [128, 2, 256]
    kvwT = chunkT(np.concatenate([Wgk_p, Wg[C:]], 0))  # [128, 2, 512]
    pwT = chunkT(proj_w)
    srwT = np.asarray(sr_w, f32).transpose(1, 2, 3, 0).reshape(2, 128, 4, C)
    srwT = np.ascontiguousarray(srwT.transpose(1, 0, 2, 3)).astype(F16NP)
    aqT = chunkT(a_q)                                  # [128, 2, 8]
    avT = chunkT(Avg)
    bqT = np.ascontiguousarray(bq_p.T.reshape(R, 2, 128)).astype(F16NP)
    bvTk = np.ascontiguousarray(bvk_p.T.reshape(R, 2, 128)).astype(F16NP)

    wbc = np.zeros((128, WBC_COLS), F16NP)
    wbc[:, OC_SRWT:OC_SRWT + 2048] = srwT.reshape(128, 2048)
    wbc[:, OC_ONE1C] = F16NP(1.0 / C)

    blob = np.zeros((128, BLOB_COLS), F16NP)
    blob[:, O_QWT:O_QWT + 512] = qwT.reshape(128, 512)
    blob[:, O_KVWT:O_KVWT + 1024] = kvwT.reshape(128, 1024)
    blob[:, O_PWT:O_PWT + 512] = pwT.reshape(128, 512)
    blob[:, O_AQT:O_AQT + 16] = aqT.reshape(128, 16)
    blob[:, O_AVT:O_AVT + 16] = avT.reshape(128, 16)
    blob[:, O_IDENT:O_IDENT + 128] = np.eye(128, dtype=F16NP)
    blob[0:R, O_BQT:O_BQT + 256] = bqT.reshape(R, 256)
    blob[0:R, O_BVTK:O_BVTK + 256] = bvTk.reshape(R, 256)
    blob[0:R, O_BVTV:O_BVTV + 256] = b_v.T.astype(F16NP)
    blob[0, O_WG1K:O_WG1K + 256] = wg1k_p.reshape(2, 128).astype(F16NP).reshape(256)
    blob[0, O_WG1V:O_WG1V + 256] = wg1[C:].astype(F16NP)
    blob[0, O_AVG1:O_AVG1 + 8] = avg1.astype(F16NP)

    def pcols(v):  # [n*128] -> [128, n]
        v = np.asarray(v, f32)
        return np.ascontiguousarray(v.reshape(-1, 128).T)

    wf = np.zeros((128, 4), f32)
    wf[:, 0:2] = pcols(qb_p)
    wf[:, 2:4] = pcols(np.asarray(sr_b, f32))
    return dict(wbc=wbc, wb=blob, wf=wf, pb=pb_eff)


def kernel(x, q_w, q_b, kv_w, kv_b, proj_w, proj_b, a_q, b_q, a_v, b_v,
           sr_w, sr_b, ln_g, ln_b, H, W):
    from concourse.bass_utils import run_bass_kernel_spmd

    x = np.asarray(x, np.float32)
    assert x.shape == (B, N, C) and int(H) == 64 and int(W) == 64

    if "nc" not in _CACHE:
        _CACHE["nc"] = _build_program()
    nc = _CACHE["nc"]

    shared = _prep_shared(q_w, q_b, kv_w, kv_b, proj_w, proj_b, a_q, b_q,
                          a_v, b_v, sr_w, sr_b, ln_g, ln_b)
    in_maps = []
    for c in range(NCORES):
        b, j = c // 4, c % 4
        xb = np.roll(x[b], -NB * j, axis=0)             # own block at rows 0:1024
        xT = np.ascontiguousarray(xb.T.astype(F16NP))   # [256, 4096]
        xT = np.ascontiguousarray(
            xT.reshape(2, 128, N).transpose(1, 0, 2))   # [128, 2, 4096]
        in_maps.append(dict(shared, xT=xT))

    res = run_bass_kernel_spmd(nc, in_maps, list(range(NCORES)))
    _CACHE["res"] = res
    out = np.empty((B, N, C), np.float32)
    for c in range(NCORES):
        b, j = c // 4, c % 4
        out[b, NB * j:NB * (j + 1)] = res.results[c]["out"]
    return out


# revision 20
# speedup vs baseline: 1.4162x; 1.0129x over previous
"""Trainium2 Bass kernel for PVT-style spatial-reduction attention (v3).

Shapes (hardcoded): x [2, 4096, 256], HEAD=8, dh=32, SR=2, R=8, H=W=64.
Sharding: core c = (batch b = c//4, query block j = c%4). Each core computes
q/attention/proj for its 1024 query rows and redundantly computes the small
conv+LN+KV path for its batch. Per-core x is pre-rotated on host so each
core's own query block is rows 0:1024.

v3 design:
- fp16 compute chain (conv/kv/q/pv/proj); fp8e4m3 DoubleRow score matmuls
  (0.5 cyc/row) via host-permuted q/k projection columns + per-head repack
  DMAs to 32-aligned lane bases.
- Transposed pv (pts stationary): 33-row pv matmuls, per-partition softmax
  denominators (single reciprocal + stride-0 scale, no broadcasts).
- V computed token-major directly (xsb stationary) - no PE transposes.
- Softmax exp split across ScalarE (true Exp) and DVE (one-op Schraudolph
  int16 bit-trick into fp16); per-head schedule interleaves the engines.
- Attention starts after strip 0: scores for kv blocks 0-3 of all heads are
  emitted interleaved with strip-1 conv/kv so ScalarE/DVE start ~15us in.
- Few large DMAs; conv weights in a separate first blob for an early start.

PSUM (8 banks): P1 2x[128,1024]f32 (4) for q-path/scores/proj, CVKV
2x[128,512]f32 (2) for conv/kv/v/transposes, PV 2x[128,8,33]f32 (2) for pv.
"""
import sys

if "/opt/trn_rl_repo" not in sys.path:
    sys.path.insert(0, "/opt/trn_rl_repo")

import numpy as np

F16NP = np.float16

HEAD, DH, C, N, B, M, R = 8, 32, 256, 4096, 2, 1024, 8
NB = N // 4          # query rows per core
SCALE = DH ** -0.5
NCORES = 8
MAGIC = 0x5F3759DF
LOG2E8 = 1477.3195879  # 2^10 / ln 2
BITB = 15317.95        # tuned fp16 Schraudolph offset (trunc semantics)

# per-head exp engine schedule over mc=0..7 (A=ScalarE exp, D=DVE bit trick);
# alternating 6A2D / 5A3D heads -> 44 ACT / 20 DVE tiles.
EXP_SCHED = ["AADAADAA", "ADAADAAD"] * 4

# conv blob (wbc): srwT 2048 | ones1c 1
OC_SRWT, OC_ONE1C = 0, 2048
WBC_COLS = 2064
# main blob (wb) column offsets (fp16 elements)
O_QWT, O_KVWT, O_PWT = 0, 512, 1536
O_AQT, O_AVT, O_IDENT, O_BQT = 2048, 2064, 2080, 2208
O_BVTK, O_BVTV, O_WG1K, O_WG1V = 2464, 2720, 2976, 3232
O_AVG1 = 3488
BLOB_COLS = 3504

_CACHE = {}
DEBUG = False


def _build_program():
    import concourse.bass as bass
    import concourse.tile as tile
    from concourse.bacc import Bacc
    from concourse import mybir

    F32 = mybir.dt.float32
    F16 = mybir.dt.float16
    FP8 = mybir.dt.float8e4
    I16 = mybir.dt.int16
    I32 = mybir.dt.int32
    U8 = mybir.dt.uint8
    AF = mybir.ActivationFunctionType
    ALU = mybir.AluOpType
    PM = mybir.MatmulPerfMode

    nc = Bacc()
    P = 128
    ST = 512       # kv tokens per strip

    def s0(ap, n):
        # stride-0 broadcast along a new innermost free dim
        return bass.AP(tensor=ap.tensor, offset=ap.offset,
                       ap=[list(d) for d in ap.ap] + [[0, n]])

    def bcast(ap, nparts):
        return bass.AP(tensor=ap.tensor, offset=ap.offset,
                       ap=[[0, nparts]] + [list(d) for d in ap.ap])

    xT_d = nc.declare_dram_parameter("xT", [P, 2, N], F16, isOutput=False)
    wbc_d = nc.declare_dram_parameter("wbc", [P, WBC_COLS], F16, isOutput=False)
    wb_d = nc.declare_dram_parameter("wb", [P, BLOB_COLS], F16, isOutput=False)
    wf_d = nc.declare_dram_parameter("wf", [P, 4], F32, isOutput=False)
    pb_d = nc.declare_dram_parameter("pb", [C], F32, isOutput=False)
    out_d = nc.declare_dram_parameter("out", [NB, C], F32, isOutput=True)
    if DEBUG:
        dbg = {
            "d_xsb": nc.declare_dram_parameter("d_xsb", [2, P, 2, 512], F16, isOutput=True),
            "d_an": nc.declare_dram_parameter("d_an", [2, P, 4], F32, isOutput=True),
            "d_k8": nc.declare_dram_parameter("d_k8", [P, 2, M], U8, isOutput=True),
            "d_q8": nc.declare_dram_parameter("d_q8", [P, 2, NB], U8, isOutput=True),
            "d_kA": nc.declare_dram_parameter("d_kA", [P, 2, M], U8, isOutput=True),
            "d_qA": nc.declare_dram_parameter("d_qA", [P, 2, NB], U8, isOutput=True),
            "d_vsb": nc.declare_dram_parameter("d_vsb", [2, P, 4, HEAD, DH + 1], F16, isOutput=True),
            "d_pts0": nc.declare_dram_parameter("d_pts0", [P, 8, NB], F16, isOutput=True),
            "d_rec": nc.declare_dram_parameter("d_rec", [HEAD, P, 8], F32, isOutput=True),
            "d_onorm": nc.declare_dram_parameter("d_onorm", [P, 8, HEAD, DH], F16, isOutput=True),
            "d_outT": nc.declare_dram_parameter("d_outT", [P, 2, NB], F16, isOutput=True),
        }

    with tile.TileContext(nc) as tc:
        with tc.tile_pool(name="wgt", bufs=1) as WGT, \
             tc.tile_pool(name="acts", bufs=1) as ACTS, \
             tc.tile_pool(name="str", bufs=2) as STR, \
             tc.tile_pool(name="tmp", bufs=2) as TMP, \
             tc.tile_pool(name="pts", bufs=8) as PTS, \
             tc.tile_pool(name="fin", bufs=2) as FIN, \
             tc.tile_pool(name="p1", bufs=2, space="PSUM") as P1, \
             tc.tile_pool(name="cvkv", bufs=2, space="PSUM") as CVKV, \
             tc.tile_pool(name="pv", bufs=2, space="PSUM") as PV, \
             tc.tile_pool(name="dscr", bufs=2, space="DRAM") as DSCR:

            # ---------------- loads (order matters for early start) ----------
            wbc = WGT.tile([P, WBC_COLS], F16, tag="wbc")
            nc.sync.dma_start(out=wbc[:], in_=wbc_d[:])
            wf = WGT.tile([P, 4], F32, tag="wf")
            nc.sync.dma_start(out=wf[:], in_=wf_d[:])
            xs0 = ACTS.tile([P, 2, 2048], F16, tag="xT0")
            nc.sync.dma_start(out=xs0[:], in_=xT_d[:, :, 0:2048])
            wb = WGT.tile([P, BLOB_COLS], F16, tag="wb")
            nc.sync.dma_start(out=wb[:], in_=wb_d[:])
            xs1 = ACTS.tile([P, 2, 2048], F16, tag="xT1")
            nc.sync.dma_start(out=xs1[:], in_=xT_d[:, :, 2048:4096])
            pbB = WGT.tile([P, C], F32, tag="pbB")
            nc.sync.dma_start(out=pbB[:], in_=bcast(pb_d.ap(), P))
            xTs = [xs0, xs1]

            srwT = wbc[:, OC_SRWT:OC_SRWT + 2048].rearrange(
                "p (ch t c) -> p ch t c", ch=2, t=4)
            ones1c = wbc[:, OC_ONE1C:OC_ONE1C + 1]
            qwT = wb[:, O_QWT:O_QWT + 512].rearrange("p (ch c) -> p ch c", ch=2)
            kvwT = wb[:, O_KVWT:O_KVWT + 1024].rearrange("p (ch c) -> p ch c", ch=2)
            pwT = wb[:, O_PWT:O_PWT + 512].rearrange("p (ch c) -> p ch c", ch=2)
            aqT = wb[:, O_AQT:O_AQT + 16].rearrange("p (ch r) -> p ch r", ch=2)
            avT = wb[:, O_AVT:O_AVT + 16].rearrange("p (ch r) -> p ch r", ch=2)
            ident = wb[:, O_IDENT:O_IDENT + 128]
            bqT = wb[0:R, O_BQT:O_BQT + 256].rearrange("r (ch c) -> r ch c", ch=2)
            bvTk = wb[0:R, O_BVTK:O_BVTK + 256].rearrange("r (ch c) -> r ch c", ch=2)
            bvTv = wb[0:R, O_BVTV:O_BVTV + 256]
            wg1k = wb[0:1, O_WG1K:O_WG1K + 256].rearrange("a (ch c) -> a ch c", ch=2)
            wg1v = wb[0:1, O_WG1V:O_WG1V + 256]
            avg1 = wb[0:1, O_AVG1:O_AVG1 + 8]

            # persistent activations
            k8tmp = ACTS.tile([P, 2, M], FP8, tag="k8tmp")
            q8tmp = ACTS.tile([P, 2, NB], FP8, tag="q8tmp")
            kO = ACTS.tile([P, 2, M], FP8, tag="kO")
            qO = ACTS.tile([P, 2, NB], FP8, tag="qO")
            outT = ACTS.tile([P, 2, NB], F16, tag="outT")
            onorm = ACTS.tile([P, 8, HEAD, DH], F16, tag="onorm")
            tq = ACTS.tile([R, NB], F16, tag="tq")

            xsbs, sqs, negmus, t2s, vsbs, ans, ascls, abits = \
                [], [], [], [], [], [], [], []

            # ---------------- per-strip pieces as emission chunks ------------
            def conv_oc(s, oc):
                xs_t = xTs[s]
                if oc == 0:
                    xsb_s = STR.tile([P, 2, ST], F16, tag="xsb")
                    xsbs.append(xsb_s)
                xsb_s = xsbs[s]
                cps = CVKV.tile([P, ST], F32, tag="cv")
                first = True
                for cc in range(2):
                    xv = xs_t[:, cc, :].rearrange(
                        "p (i a j b) -> p i a j b", i=16, a=2, j=32, b=2)
                    for di in range(2):
                        for dj in range(2):
                            nc.tensor.matmul(
                                cps[:], srwT[:, cc, di * 2 + dj,
                                             oc * P:(oc + 1) * P],
                                xv[:, :, di, :, dj],
                                start=first,
                                stop=(cc == 1 and di == 1 and dj == 1))
                            first = False
                if s == 0:
                    nc.scalar.activation(out=xsb_s[:, oc, :], in_=cps[:],
                                         func=AF.Identity,
                                         bias=wf[:, 2 + oc:3 + oc])
                else:
                    nc.vector.tensor_scalar_add(
                        out=xsb_s[:, oc, :], in0=cps[:], scalar1=wf[:, 2 + oc:3 + oc])
                if DEBUG and oc == 1:
                    nc.sync.dma_start(out=dbg["d_xsb"][s], in_=xsb_s[:])

            def stats(s):
                xsb_s = xsbs[s]
                sq_s = STR.tile([P, 2, ST], F16, tag="sq")
                nc.gpsimd.tensor_mul(out=sq_s[:], in0=xsb_s[:], in1=xsb_s[:])
                sxp = CVKV.tile([1, ST], F32, tag="cv")
                nc.tensor.matmul(sxp[:], ones1c, xsb_s[:, 0, :], start=True, stop=False)
                nc.tensor.matmul(sxp[:], ones1c, xsb_s[:, 1, :], start=False, stop=True)
                negmu = TMP.tile([1, ST], F16, tag="negmu")
                nc.vector.tensor_scalar_mul(out=negmu[:], in0=sxp[:], scalar1=-1.0)
                negmus.append(negmu)
                sxxp = CVKV.tile([1, ST], F32, tag="cv")
                nc.tensor.matmul(sxxp[:], ones1c, sq_s[:, 0, :], start=True, stop=False)
                nc.tensor.matmul(sxxp[:], ones1c, sq_s[:, 1, :], start=False, stop=True)
                ex2_sb = TMP.tile([1, ST], F32, tag="ex2sb")
                nc.vector.tensor_copy(out=ex2_sb[:], in_=sxxp[:])

                # chunk-major repack [1,512] -> [128,4] via DRAM bounce
                nm_d = DSCR.tile([ST], F16, tag=f"nm{s}")
                nc.sync.dma_start(out=nm_d[:], in_=negmu[:])
                ex_d = DSCR.tile([ST], F32, tag=f"ex{s}")
                nc.sync.dma_start(out=ex_d[:], in_=ex2_sb[:])
                mur = TMP.tile([P, 4], F16, tag="mur")
                nc.sync.dma_start(out=mur[:],
                                  in_=nm_d[:].rearrange("(g p) -> p g", p=P))
                ex2r = TMP.tile([P, 4], F32, tag="ex2r")
                nc.sync.dma_start(out=ex2r[:],
                                  in_=ex_d[:].rearrange("(g p) -> p g", p=P))

                # rstd via quake rsqrt (1 newton), [128,4] chain on DVE
                nmu2 = TMP.tile([P, 4], F32, tag="nmu2")
                nc.vector.scalar_tensor_tensor(out=nmu2[:], in0=mur[:], scalar=-1.0,
                                               in1=mur[:], op0=ALU.mult, op1=ALU.mult)
                ve = TMP.tile([P, 4], F32, tag="ve")
                nc.vector.scalar_tensor_tensor(out=ve[:], in0=nmu2[:], scalar=1e-5,
                                               in1=ex2r[:], op0=ALU.add, op1=ALU.add)
                hsh = TMP.tile([P, 4], I32, tag="hsh")
                nc.vector.tensor_scalar(out=hsh[:], in0=ve[:].bitcast(I32), scalar1=1,
                                        scalar2=None, op0=ALU.logical_shift_right)
                nc.vector.tensor_scalar(out=hsh[:], in0=hsh[:], scalar1=-1,
                                        scalar2=MAGIC, op0=ALU.mult, op1=ALU.add)
                y0 = hsh[:].bitcast(F32)
                nt = TMP.tile([P, 4], F32, tag="nt")
                nc.vector.tensor_mul(out=nt[:], in0=y0, in1=y0)
                nc.vector.scalar_tensor_tensor(out=nt[:], in0=nt[:], scalar=-0.5,
                                               in1=ve[:], op0=ALU.mult, op1=ALU.mult)
                nc.vector.tensor_scalar_add(out=nt[:], in0=nt[:], scalar1=1.5)
                an_s = STR.tile([P, 4], F32, tag="an")
                nc.vector.tensor_mul(out=an_s[:], in0=y0, in1=nt[:])
                ascl_s = STR.tile([P, 4], F32, tag="ascl")
                nc.vector.tensor_scalar_mul(out=ascl_s[:], in0=an_s[:], scalar1=SCALE)
                abit_s = STR.tile([P, 4], F32, tag="abit")
                nc.vector.tensor_scalar_mul(out=abit_s[:], in0=ascl_s[:],
                                            scalar1=LOG2E8)
                ans.append(an_s)
                ascls.append(ascl_s)
                abits.append(abit_s)
                if DEBUG:
                    nc.sync.dma_start(out=dbg["d_an"][s], in_=an_s[:])

            def t2_and_k(s):
                xsb_s, negmu = xsbs[s], negmus[s]
                t2p = CVKV.tile([R, ST], F32, tag="cv")
                nc.tensor.matmul(t2p[:], avT[:, 0, :], xsb_s[:, 0, :], start=True, stop=False)
                nc.tensor.matmul(t2p[:], avT[:, 1, :], xsb_s[:, 1, :], start=False, stop=False)
                nc.tensor.matmul(t2p[:], avg1, negmu[:], start=False, stop=True)
                t2 = TMP.tile([R, ST], F16, tag="t2")
                if s == 0:
                    nc.scalar.copy(out=t2[:], in_=t2p[:])
                else:
                    nc.vector.tensor_copy(out=t2[:], in_=t2p[:])
                t2s.append(t2)
                for kvoc in range(2):
                    kps = CVKV.tile([P, ST], F32, tag="cv")
                    nc.tensor.matmul(kps[:], kvwT[:, 0, kvoc * P:(kvoc + 1) * P],
                                     xsb_s[:, 0, :], start=True, stop=False)
                    nc.tensor.matmul(kps[:], kvwT[:, 1, kvoc * P:(kvoc + 1) * P],
                                     xsb_s[:, 1, :], start=False, stop=False)
                    nc.tensor.matmul(kps[:], wg1k[:, kvoc, :], negmu[:],
                                     start=False, stop=False)
                    nc.tensor.matmul(kps[:], bvTk[:, kvoc, :], t2[:],
                                     start=False, stop=True)
                    if s == 0:
                        nc.scalar.copy(out=k8tmp[:, kvoc, s * ST:(s + 1) * ST],
                                       in_=kps[:])
                    else:
                        nc.vector.tensor_copy(
                            out=k8tmp[:, kvoc, s * ST:(s + 1) * ST], in_=kps[:])

            def k_repack(s):
                # odd heads h=2a+1 (packed base 16h=32a+16) -> kO base 32a
                for a in range(4):
                    h = 2 * a + 1
                    nc.gpsimd.dma_start(
                        out=kO[32 * a:32 * a + 16, :, s * ST:(s + 1) * ST],
                        in_=k8tmp[16 * h:16 * h + 16, :, s * ST:(s + 1) * ST])

            def v_path(s):
                # token-major v: out[m, vchan] via xsb/negmu/t2 as stationary
                xsb_s, negmu, t2, an_s = xsbs[s], negmus[s], t2s[s], ans[s]
                vsb_s = STR.tile([P, 4, HEAD, DH + 1], F16, tag="vsb")
                for ml in range(4):
                    sl = slice(ml * P, (ml + 1) * P)
                    vP = CVKV.tile([P, C], F32, tag="cv")
                    nc.tensor.matmul(vP[:], xsb_s[:, 0, sl], kvwT[:, 0, 256:512],
                                     start=True, stop=False)
                    nc.tensor.matmul(vP[:], xsb_s[:, 1, sl], kvwT[:, 1, 256:512],
                                     start=False, stop=False)
                    nc.tensor.matmul(vP[:], negmu[:, sl], wg1v,
                                     start=False, stop=False, tile_position=(0, 0))
                    nc.tensor.matmul(vP[:], t2[:, sl], bvTv,
                                     start=False, stop=True, tile_position=(0, 0))
                    if s == 0:
                        nc.scalar.activation(
                            out=vsb_s[:, ml, :, 0:DH],
                            in_=vP[:].rearrange("p (h d) -> p h d", d=DH),
                            func=AF.Copy, scale=an_s[:, ml:ml + 1])
                    else:
                        nc.vector.tensor_scalar_mul(
                            out=vsb_s[:, ml, :, 0:DH],
                            in0=vP[:].rearrange("p (h d) -> p h d", d=DH),
                            scalar1=an_s[:, ml:ml + 1])
                nc.gpsimd.memset(vsb_s[:, :, :, DH:DH + 1], 1.0)
                vsbs.append(vsb_s)
                if DEBUG:
                    nc.sync.dma_start(out=dbg["d_vsb"][s], in_=vsb_s[:])

            def q_path():
                xs_t = xTs[0]
                tqp = P1.tile([R, NB], F32, tag="p1")
                for nh in range(2):
                    sl = slice(nh * 512, (nh + 1) * 512)
                    nc.tensor.matmul(tqp[:, sl], aqT[:, 0, :], xs_t[:, 0, sl],
                                     start=True, stop=False)
                    nc.tensor.matmul(tqp[:, sl], aqT[:, 1, :], xs_t[:, 1, sl],
                                     start=False, stop=True)
                nc.vector.tensor_copy(out=tq[:], in_=tqp[:])
                for oc in range(2):
                    qps = P1.tile([P, NB], F32, tag="p1")
                    for nh in range(2):
                        sl = slice(nh * 512, (nh + 1) * 512)
                        nc.tensor.matmul(qps[:, sl],
                                         qwT[:, 0, oc * P:(oc + 1) * P],
                                         xs_t[:, 0, sl], start=True, stop=False)
                        nc.tensor.matmul(qps[:, sl],
                                         qwT[:, 1, oc * P:(oc + 1) * P],
                                         xs_t[:, 1, sl], start=False, stop=False)
                        nc.tensor.matmul(qps[:, sl], bqT[:, oc, :], tq[:, sl],
                                         start=False, stop=True)
                    nc.scalar.activation(out=q8tmp[:, oc, :], in_=qps[:],
                                         func=AF.Identity,
                                         bias=wf[:, oc:oc + 1])
                for a in range(4):
                    h = 2 * a + 1
                    nc.sync.dma_start(
                        out=qO[32 * a:32 * a + 16],
                        in_=q8tmp[16 * h:16 * h + 16])

            # ---------------- attention pieces ----------------
            ptshs = [None] * HEAD

            def score_pair(h, mcp):
                # scores+exp for mc = 2*mcp, 2*mcp+1; even heads read the
                # packed tiles directly (base 16h is 32-aligned), odd heads
                # the repacked kO/qO
                if h % 2 == 0:
                    a, kT, qT8 = h // 2, k8tmp, q8tmp
                else:
                    a, kT, qT8 = h // 2, kO, qO
                if ptshs[h] is None:
                    ptsh = PTS.tile([P, 8, NB], F16, tag="ptsh")
                    ptshs[h] = ptsh
                ptsh = ptshs[h]
                for mc in (2 * mcp, 2 * mcp + 1):
                    s_, ml = mc // 4, mc % 4
                    stile = P1.tile([P, NB], F32, tag="p1")
                    for qc in range(4):
                        nc.tensor.matmul(
                            stile[:, qc * 256:(qc + 1) * 256],
                            kT[32 * a:32 * a + 16, :, mc * P:(mc + 1) * P],
                            qT8[32 * a:32 * a + 16, :, qc * 256:(qc + 1) * 256],
                            start=True, stop=True, perf_mode=PM.DoubleRow,
                            tile_position=(32 * a, 0))
                    if EXP_SCHED[h][mc] == "A":
                        nc.scalar.activation(out=ptsh[:, mc, :], in_=stile[:],
                                             func=AF.Exp,
                                             scale=ascls[s_][:, ml:ml + 1])
                    else:
                        nc.vector.tensor_scalar(
                            out=ptsh[:, mc, :].bitcast(I16), in0=stile[:],
                            scalar1=abits[s_][:, ml:ml + 1], scalar2=BITB,
                            op0=ALU.mult, op1=ALU.add)

            def pv_head(h):
                ptsh = ptshs[h]
                pvq = PV.tile([P, 8, DH + 1], F32, tag="pv")
                for qb in range(8):
                    for mc in range(8):
                        nc.tensor.matmul(
                            pvq[:, qb, :],
                            ptsh[:, mc, qb * P:(qb + 1) * P],
                            vsbs[mc // 4][:, mc % 4, h, :],
                            start=(mc == 0), stop=(mc == 7),
                            tile_position=(0, 0))
                rec = TMP.tile([P, 8], F32, tag="rec")
                nc.vector.reciprocal(out=rec[:], in_=pvq[:, :, DH])
                nc.vector.tensor_tensor(out=onorm[:, :, h, :],
                                        in0=pvq[:, :, 0:DH],
                                        in1=s0(rec[:], DH), op=ALU.mult)
                if DEBUG:
                    if h == 0:
                        nc.sync.dma_start(out=dbg["d_pts0"][:], in_=ptsh[:])
                    nc.sync.dma_start(out=dbg["d_rec"][h], in_=rec[:])

            def transpose_ch(ch):
                for qb in range(8):
                    ttile = CVKV.tile([P, P], F16, tag="cv")
                    nc.tensor.transpose(
                        ttile[:],
                        onorm[:, qb, ch * 4:(ch + 1) * 4, :].rearrange(
                            "p h d -> p (h d)"),
                        ident)
                    nc.vector.tensor_copy(
                        out=outT[:, ch, qb * P:(qb + 1) * P], in_=ttile[:])

            # ---------------- emission schedule ----------------
            conv_oc(0, 0)
            conv_oc(0, 1)
            stats(0)
            t2_and_k(0)
            k_repack(0)
            q_path()
            v_path(0)

            # interleave strip 1 with early scores (all heads, kv blocks 0-3)
            s1_chunks = [
                lambda: conv_oc(1, 0),
                lambda: conv_oc(1, 1),
                lambda: stats(1),
                lambda: t2_and_k(1),
                lambda: (k_repack(1), v_path(1)),
            ]
            b1_chunks = [(h, mcp) for h in (0, 2, 4, 6, 1, 3, 5, 7)
                         for mcp in (0, 1)]
            bi = 0
            for chunk in s1_chunks:
                chunk()
                for _ in range(2):
                    if bi < len(b1_chunks):
                        score_pair(*b1_chunks[bi])
                        bi += 1
            while bi < len(b1_chunks):
                score_pair(*b1_chunks[bi])
                bi += 1

            # second half: kv blocks 4-7 per head, pv pipelined one head behind
            for h in range(HEAD):
                score_pair(h, 2)
                score_pair(h, 3)
                if h > 0:
                    pv_head(h - 1)
                if h == 4:
                    transpose_ch(0)   # heads 0-3 normalized by now
            pv_head(HEAD - 1)
            if DEBUG:
                nc.sync.dma_start(out=dbg["d_onorm"][:], in_=onorm[:])
            transpose_ch(1)
            if DEBUG:
                nc.sync.dma_start(out=dbg["d_outT"][:], in_=outT[:])

            # ---------------- projection (per-t8, pipelined) ----------------
            for t8 in range(8):
                pp = P1.tile([P, C], F32, tag="p1")
                nc.tensor.matmul(pp[:], outT[:, 0, t8 * P:(t8 + 1) * P],
                                 pwT[:, 0, :], start=True, stop=False)
                nc.tensor.matmul(pp[:], outT[:, 1, t8 * P:(t8 + 1) * P],
                                 pwT[:, 1, :], start=False, stop=True)
                fin = FIN.tile([P, C], F32, tag="fin")
                nc.vector.tensor_add(out=fin[:], in0=pp[:], in1=pbB[:])
                nc.sync.dma_start(out=out_d[t8 * P:(t8 + 1) * P], in_=fin[:])

    nc.finalize()
    return nc


P_ = 128


def _kq_perm():
    # PSUM chunk oc holds half `oc` of all heads: partition p=16h+lane ->
    # original channel 32h + 16*oc + lane
    perm = np.empty(2 * P_, np.int64)
    for oc in range(2):
        for p in range(P_):
            perm[oc * P_ + p] = 32 * (p // 16) + 16 * oc + (p % 16)
    return perm


def _prep_shared(q_w, q_b, kv_w, kv_b, proj_w, proj_b, a_q, b_q, a_v, b_v,
                 sr_w, sr_b, ln_g, ln_b):
    f32 = np.float32

    def chunkT(w):  # [out, in] -> [128, n_in_chunks, out]
        wt = np.ascontiguousarray(np.asarray(w, f32).T)
        ic, oc = wt.shape
        return np.ascontiguousarray(
            wt.reshape(ic // 128, 128, oc).transpose(1, 0, 2)).astype(F16NP)

    kv_w = np.asarray(kv_w, f32)
    a_v = np.asarray(a_v, f32)
    b_v = np.asarray(b_v, f32)
    b_q = np.asarray(b_q, f32)
    q_w = np.asarray(q_w, f32)
    q_b = np.asarray(q_b, f32)
    g = np.asarray(ln_g, f32)
    bb = np.asarray(ln_b, f32)
    proj_w = np.asarray(proj_w, f32)

    # fold LayerNorm gamma into kv/a_v weights; mean via rank-1 correction;
    # k-side constants dropped (softmax shift invariance), v-side constants
    # folded into the projection bias.
    Wg = kv_w * g[None, :]
    wg1 = Wg.sum(1)
    Avg = a_v * g[None, :]
    avg1 = Avg.sum(1)
    wbt = kv_w @ bb + np.asarray(kv_b, f32)
    dconst = b_v @ (a_v @ bb)
    wv_const = wbt[C:] + dconst
    pb_eff = np.asarray(proj_b, f32) + proj_w @ wv_const

    perm = _kq_perm()
    qw_p = q_w[perm]
    qb_p = q_b[perm]
    bq_p = b_q[perm]
    Wgk_p = Wg[0:C][perm]
    wg1k_p = wg1[0:C][perm]
    bvk_p = b_v[perm]

    qwT = chunkT(qw_p)                                 # [128, 2, 256]
    kvwT = chunkT(np.concatenate([Wgk_p, Wg[C:]], 0))  # [128, 2, 512]
    pwT = chunkT(proj_w)
    srwT = np.asarray(sr_w, f32).transpose(1, 2, 3, 0).reshape(2, 128, 4, C)
    srwT = np.ascontiguousarray(srwT.transpose(1, 0, 2, 3)).astype(F16NP)
    aqT = chunkT(a_q)                                  # [128, 2, 8]
    avT = chunkT(Avg)
    bqT = np.ascontiguousarray(bq_p.T.reshape(R, 2, 128)).astype(F16NP)
    bvTk = np.ascontiguousarray(bvk_p.T.reshape(R, 2, 128)).astype(F16NP)

    wbc = np.zeros((128, WBC_COLS), F16NP)
    wbc[:, OC_SRWT:OC_SRWT + 2048] = srwT.reshape(128, 2048)
    wbc[:, OC_ONE1C] = F16NP(1.0 / C)

    blob = np.zeros((128, BLOB_COLS), F16NP)
    blob[:, O_QWT:O_QWT + 512] = qwT.reshape(128, 512)
    blob[:, O_KVWT:O_KVWT + 1024] = kvwT.reshape(128, 1024)
    blob[:, O_PWT:O_PWT + 512] = pwT.reshape(128, 512)
    blob[:, O_AQT:O_AQT + 16] = aqT.reshape(128, 16)
    blob[:, O_AVT:O_AVT + 16] = avT.reshape(128, 16)
    blob[:, O_IDENT:O_IDENT + 128] = np.eye(128, dtype=F16NP)
    blob[0:R, O_BQT:O_BQT + 256] = bqT.reshape(R, 256)
    blob[0:R, O_BVTK:O_BVTK + 256] = bvTk.reshape(R, 256)
    blob[0:R, O_BVTV:O_BVTV + 256] = b_v.T.astype(F16NP)
    blob[0, O_WG1K:O_WG1K + 256] = wg1k_p.reshape(2, 128).astype(F16NP).reshape(256)
    blob[0, O_WG1V:O_WG1V + 256] = wg1[C:].astype(F16NP)
    blob[0, O_AVG1:O_AVG1 + 8] = avg1.astype(F16NP)

    def pcols(v):  # [n*128] -> [128, n]
        v = np.asarray(v, f32)
        return np.ascontiguousarray(v.reshape(-1, 128).T)

    wf = np.zeros((128, 4), f32)
    wf[:, 0:2] = pcols(qb_p)
    wf[:, 2:4] = pcols(np.asarray(sr_b, f32))
    return dict(wbc=wbc, wb=blob, wf=wf, pb=pb_eff)


def kernel(x, q_w, q_b, kv_w, kv_b, proj_w, proj_b, a_q, b_q, a_v, b_v,
           sr_w, sr_b, ln_g, ln_b, H, W):
    from concourse.bass_utils import run_bass_kernel_spmd

    x = np.asarray(x, np.float32)
    assert x.shape == (B, N, C) and int(H) == 64 and int(W) == 64

    if "nc" not in _CACHE:
        _CACHE["nc"] = _build_program()
    nc = _CACHE["nc"]

    shared = _prep_shared(q_w, q_b, kv_w, kv_b, proj_w, proj_b, a_q, b_q,
                          a_v, b_v, sr_w, sr_b, ln_g, ln_b)
    in_maps = []
    for c in range(NCORES):
        b, j = c // 4, c % 4
        xb = np.roll(x[b], -NB * j, axis=0)             # own block at rows 0:1024
        xT = np.ascontiguousarray(xb.T.astype(F16NP))   # [256, 4096]
        xT = np.ascontiguousarray(
            xT.reshape(2, 128, N).transpose(1, 0, 2))   # [128, 2, 4096]
        in_maps.append(dict(shared, xT=xT))

    res = run_bass_kernel_spmd(nc, in_maps, list(range(NCORES)))
    _CACHE["res"] = res
    out = np.empty((B, N, C), np.float32)
    for c in range(NCORES):
        b, j = c // 4, c % 4
        out[b, NB * j:NB * (j + 1)] = res.results[c]["out"]
    return out


# revision 21
# speedup vs baseline: 1.6501x; 1.1651x over previous
"""Trainium2 Bass kernel for PVT-style spatial-reduction attention (v3).

Shapes (hardcoded): x [2, 4096, 256], HEAD=8, dh=32, SR=2, R=8, H=W=64.
Sharding: core c = (batch b = c//4, query block j = c%4). Each core computes
q/attention/proj for its 1024 query rows and redundantly computes the small
conv+LN+KV path for its batch. Per-core x is pre-rotated on host so each
core's own query block is rows 0:1024.

v3 design:
- fp16 compute chain (conv/kv/q/pv/proj); fp8e4m3 DoubleRow score matmuls
  (0.5 cyc/row) via host-permuted q/k projection columns + per-head repack
  DMAs to 32-aligned lane bases.
- Transposed pv (pts stationary): 33-row pv matmuls, per-partition softmax
  denominators (single reciprocal + stride-0 scale, no broadcasts).
- V computed token-major directly (xsb stationary) - no PE transposes.
- Softmax exp split across ScalarE (true Exp) and DVE (one-op Schraudolph
  int16 bit-trick into fp16); per-head schedule interleaves the engines.
- Attention starts after strip 0: scores for kv blocks 0-3 of all heads are
  emitted interleaved with strip-1 conv/kv so ScalarE/DVE start ~15us in.
- Few large DMAs; conv weights in a separate first blob for an early start.

PSUM (8 banks): P1 2x[128,1024]f32 (4) for q-path/scores/proj, CVKV
2x[128,512]f32 (2) for conv/kv/v/transposes, PV 2x[128,8,33]f32 (2) for pv.
"""
import sys

if "/opt/trn_rl_repo" not in sys.path:
    sys.path.insert(0, "/opt/trn_rl_repo")

import numpy as np

F16NP = np.float16

HEAD, DH, C, N, B, M, R = 8, 32, 256, 4096, 2, 1024, 8
NB = N // 4          # query rows per core
SCALE = DH ** -0.5
NCORES = 8
MAGIC = 0x5F3759DF
LOG2E8 = 1477.3195879  # 2^10 / ln 2
BITB = 15317.95        # tuned fp16 Schraudolph offset (trunc semantics)

# per-head exp engine schedule over mc=0..7 (A=ScalarE exp, D=DVE bit trick);
# alternating 6A2D / 5A3D heads -> 44 ACT / 20 DVE tiles.
EXP_SCHED = ["AADAADAA"] * 5 + ["ADAADAAD"] * 3

# conv blob (wbc): srwT 2048 | ones1c 1
OC_SRWT, OC_ONE1C = 0, 2048
WBC_COLS = 2064
# main blob (wb) column offsets (fp16 elements)
O_QWT, O_KVWT, O_PWT = 0, 512, 1536
O_AQT, O_AVT, O_IDENT, O_BQT = 2048, 2064, 2080, 2208
O_BVTK, O_BVTV, O_WG1K, O_WG1V = 2464, 2720, 2976, 3232
O_AVG1 = 3488
BLOB_COLS = 3504

_CACHE = {}
DEBUG = False


def _build_program():
    import concourse.bass as bass
    import concourse.tile as tile
    from concourse.bacc import Bacc
    from concourse import mybir

    F32 = mybir.dt.float32
    F16 = mybir.dt.float16
    FP8 = mybir.dt.float8e4
    I16 = mybir.dt.int16
    I32 = mybir.dt.int32
    U8 = mybir.dt.uint8
    AF = mybir.ActivationFunctionType
    ALU = mybir.AluOpType
    PM = mybir.MatmulPerfMode

    nc = Bacc()
    P = 128
    ST = 512       # kv tokens per strip

    def s0(ap, n):
        # stride-0 broadcast along a new innermost free dim
        return bass.AP(tensor=ap.tensor, offset=ap.offset,
                       ap=[list(d) for d in ap.ap] + [[0, n]])

    def bcast(ap, nparts):
        return bass.AP(tensor=ap.tensor, offset=ap.offset,
                       ap=[[0, nparts]] + [list(d) for d in ap.ap])

    xT_d = nc.declare_dram_parameter("xT", [P, 2, N], F16, isOutput=False)
    wbc_d = nc.declare_dram_parameter("wbc", [P, WBC_COLS], F16, isOutput=False)
    wb_d = nc.declare_dram_parameter("wb", [P, BLOB_COLS], F16, isOutput=False)
    wf_d = nc.declare_dram_parameter("wf", [P, 4], F32, isOutput=False)
    pb_d = nc.declare_dram_parameter("pb", [C], F32, isOutput=False)
    out_d = nc.declare_dram_parameter("out", [NB, C], F32, isOutput=True)
    if DEBUG:
        dbg = {
            "d_xsb": nc.declare_dram_parameter("d_xsb", [2, P, 2, 512], F16, isOutput=True),
            "d_an": nc.declare_dram_parameter("d_an", [2, P, 4], F32, isOutput=True),
            "d_k8": nc.declare_dram_parameter("d_k8", [P, 2, M], U8, isOutput=True),
            "d_q8": nc.declare_dram_parameter("d_q8", [P, 2, NB], U8, isOutput=True),
            "d_kA": nc.declare_dram_parameter("d_kA", [P, 2, M], U8, isOutput=True),
            "d_qA": nc.declare_dram_parameter("d_qA", [P, 2, NB], U8, isOutput=True),
            "d_vsb": nc.declare_dram_parameter("d_vsb", [2, P, 4, HEAD, DH + 1], F16, isOutput=True),
            "d_pts0": nc.declare_dram_parameter("d_pts0", [P, 8, NB], F16, isOutput=True),
            "d_rec": nc.declare_dram_parameter("d_rec", [HEAD, P, 8], F32, isOutput=True),
            "d_onorm": nc.declare_dram_parameter("d_onorm", [P, 8, HEAD, DH], F16, isOutput=True),
            "d_outT": nc.declare_dram_parameter("d_outT", [P, 2, NB], F16, isOutput=True),
        }

    with tile.TileContext(nc) as tc:
        with tc.tile_pool(name="wgt", bufs=1) as WGT, \
             tc.tile_pool(name="acts", bufs=1) as ACTS, \
             tc.tile_pool(name="str", bufs=2) as STR, \
             tc.tile_pool(name="tmp", bufs=2) as TMP, \
             tc.tile_pool(name="pts", bufs=8) as PTS, \
             tc.tile_pool(name="fin", bufs=8) as FIN, \
             tc.tile_pool(name="p1", bufs=3, space="PSUM") as P1, \
             tc.tile_pool(name="cvkv", bufs=2, space="PSUM") as CVKV, \
             tc.tile_pool(name="dscr", bufs=2, space="DRAM") as DSCR:

            # ---------------- loads (order matters for early start) ----------
            wbc = WGT.tile([P, WBC_COLS], F16, tag="wbc")
            nc.sync.dma_start(out=wbc[:], in_=wbc_d[:])
            wf = WGT.tile([P, 4], F32, tag="wf")
            nc.sync.dma_start(out=wf[:], in_=wf_d[:])
            xs0 = ACTS.tile([P, 2, 2048], F16, tag="xT0")
            nc.sync.dma_start(out=xs0[:], in_=xT_d[:, :, 0:2048])
            wb = WGT.tile([P, BLOB_COLS], F16, tag="wb")
            nc.sync.dma_start(out=wb[:], in_=wb_d[:])
            xs1 = ACTS.tile([P, 2, 2048], F16, tag="xT1")
            nc.sync.dma_start(out=xs1[:], in_=xT_d[:, :, 2048:4096])
            pbB = WGT.tile([P, C], F32, tag="pbB")
            nc.sync.dma_start(out=pbB[:], in_=bcast(pb_d.ap(), P))
            xTs = [xs0, xs1]

            srwT = wbc[:, OC_SRWT:OC_SRWT + 2048].rearrange(
                "p (ch t c) -> p ch t c", ch=2, t=4)
            ones1c = wbc[:, OC_ONE1C:OC_ONE1C + 1]
            qwT = wb[:, O_QWT:O_QWT + 512].rearrange("p (ch c) -> p ch c", ch=2)
            kvwT = wb[:, O_KVWT:O_KVWT + 1024].rearrange("p (ch c) -> p ch c", ch=2)
            pwT = wb[:, O_PWT:O_PWT + 512].rearrange("p (ch c) -> p ch c", ch=2)
            aqT = wb[:, O_AQT:O_AQT + 16].rearrange("p (ch r) -> p ch r", ch=2)
            avT = wb[:, O_AVT:O_AVT + 16].rearrange("p (ch r) -> p ch r", ch=2)
            ident = wb[:, O_IDENT:O_IDENT + 128]
            bqT = wb[0:R, O_BQT:O_BQT + 256].rearrange("r (ch c) -> r ch c", ch=2)
            bvTk = wb[0:R, O_BVTK:O_BVTK + 256].rearrange("r (ch c) -> r ch c", ch=2)
            bvTv = wb[0:R, O_BVTV:O_BVTV + 256]
            wg1k = wb[0:1, O_WG1K:O_WG1K + 256].rearrange("a (ch c) -> a ch c", ch=2)
            wg1v = wb[0:1, O_WG1V:O_WG1V + 256]
            avg1 = wb[0:1, O_AVG1:O_AVG1 + 8]

            # persistent activations
            k8tmp = ACTS.tile([P, 2, M], FP8, tag="k8tmp")
            q8tmp = ACTS.tile([P, 2, NB], FP8, tag="q8tmp")
            kO = ACTS.tile([P, 2, M], FP8, tag="kO")
            qO = ACTS.tile([P, 2, NB], FP8, tag="qO")
            outT = ACTS.tile([P, 2, NB], F16, tag="outT")
            onorm = ACTS.tile([P, 8, HEAD, DH], F16, tag="onorm")
            tq = ACTS.tile([R, NB], F16, tag="tq")

            xsbs, sqs, negmus, t2s, vsbs, ans, ascls, abits = \
                [], [], [], [], [], [], [], []

            # ---------------- per-strip pieces as emission chunks ------------
            def conv_oc(s, oc):
                xs_t = xTs[s]
                if oc == 0:
                    xsb_s = STR.tile([P, 2, ST], F16, tag="xsb")
                    xsbs.append(xsb_s)
                xsb_s = xsbs[s]
                cps = CVKV.tile([P, ST], F32, tag="cv")
                first = True
                for cc in range(2):
                    xv = xs_t[:, cc, :].rearrange(
                        "p (i a j b) -> p i a j b", i=16, a=2, j=32, b=2)
                    for di in range(2):
                        for dj in range(2):
                            nc.tensor.matmul(
                                cps[:], srwT[:, cc, di * 2 + dj,
                                             oc * P:(oc + 1) * P],
                                xv[:, :, di, :, dj],
                                start=first,
                                stop=(cc == 1 and di == 1 and dj == 1))
                            first = False
                if s == 0:
                    nc.scalar.activation(out=xsb_s[:, oc, :], in_=cps[:],
                                         func=AF.Identity,
                                         bias=wf[:, 2 + oc:3 + oc])
                else:
                    nc.vector.tensor_scalar_add(
                        out=xsb_s[:, oc, :], in0=cps[:], scalar1=wf[:, 2 + oc:3 + oc])
                if DEBUG and oc == 1:
                    nc.sync.dma_start(out=dbg["d_xsb"][s], in_=xsb_s[:])

            def stats(s):
                xsb_s = xsbs[s]
                sq_s = STR.tile([P, 2, ST], F16, tag="sq")
                nc.gpsimd.tensor_mul(out=sq_s[:], in0=xsb_s[:], in1=xsb_s[:])
                sxp = CVKV.tile([1, ST], F32, tag="cv")
                nc.tensor.matmul(sxp[:], ones1c, xsb_s[:, 0, :], start=True, stop=False)
                nc.tensor.matmul(sxp[:], ones1c, xsb_s[:, 1, :], start=False, stop=True)
                negmu = TMP.tile([1, ST], F16, tag="negmu")
                nc.vector.tensor_scalar_mul(out=negmu[:], in0=sxp[:], scalar1=-1.0)
                negmus.append(negmu)
                sxxp = CVKV.tile([1, ST], F32, tag="cv")
                nc.tensor.matmul(sxxp[:], ones1c, sq_s[:, 0, :], start=True, stop=False)
                nc.tensor.matmul(sxxp[:], ones1c, sq_s[:, 1, :], start=False, stop=True)
                ex2_sb = TMP.tile([1, ST], F32, tag="ex2sb")
                nc.vector.tensor_copy(out=ex2_sb[:], in_=sxxp[:])

                # chunk-major repack [1,512] -> [128,4] via DRAM bounce
                nm_d = DSCR.tile([ST], F16, tag=f"nm{s}")
                nc.sync.dma_start(out=nm_d[:], in_=negmu[:])
                ex_d = DSCR.tile([ST], F32, tag=f"ex{s}")
                nc.sync.dma_start(out=ex_d[:], in_=ex2_sb[:])
                mur = TMP.tile([P, 4], F16, tag="mur")
                nc.sync.dma_start(out=mur[:],
                                  in_=nm_d[:].rearrange("(g p) -> p g", p=P))
                ex2r = TMP.tile([P, 4], F32, tag="ex2r")
                nc.sync.dma_start(out=ex2r[:],
                                  in_=ex_d[:].rearrange("(g p) -> p g", p=P))

                # rstd via quake rsqrt (1 newton), [128,4] chain on DVE
                nmu2 = TMP.tile([P, 4], F32, tag="nmu2")
                nc.vector.scalar_tensor_tensor(out=nmu2[:], in0=mur[:], scalar=-1.0,
                                               in1=mur[:], op0=ALU.mult, op1=ALU.mult)
                ve = TMP.tile([P, 4], F32, tag="ve")
                nc.vector.scalar_tensor_tensor(out=ve[:], in0=nmu2[:], scalar=1e-5,
                                               in1=ex2r[:], op0=ALU.add, op1=ALU.add)
                hsh = TMP.tile([P, 4], I32, tag="hsh")
                nc.vector.tensor_scalar(out=hsh[:], in0=ve[:].bitcast(I32), scalar1=1,
                                        scalar2=None, op0=ALU.logical_shift_right)
                nc.vector.tensor_scalar(out=hsh[:], in0=hsh[:], scalar1=-1,
                                        scalar2=MAGIC, op0=ALU.mult, op1=ALU.add)
                y0 = hsh[:].bitcast(F32)
                nt = TMP.tile([P, 4], F32, tag="nt")
                nc.vector.tensor_mul(out=nt[:], in0=y0, in1=y0)
                nc.vector.scalar_tensor_tensor(out=nt[:], in0=nt[:], scalar=-0.5,
                                               in1=ve[:], op0=ALU.mult, op1=ALU.mult)
                nc.vector.tensor_scalar_add(out=nt[:], in0=nt[:], scalar1=1.5)
                an_s = STR.tile([P, 4], F32, tag="an")
                nc.vector.tensor_mul(out=an_s[:], in0=y0, in1=nt[:])
                ascl_s = STR.tile([P, 4], F32, tag="ascl")
                nc.vector.tensor_scalar_mul(out=ascl_s[:], in0=an_s[:], scalar1=SCALE)
                abit_s = STR.tile([P, 4], F32, tag="abit")
                nc.vector.tensor_scalar_mul(out=abit_s[:], in0=ascl_s[:],
                                            scalar1=LOG2E8)
                ans.append(an_s)
                ascls.append(ascl_s)
                abits.append(abit_s)
                if DEBUG:
                    nc.sync.dma_start(out=dbg["d_an"][s], in_=an_s[:])

            def t2_and_k(s):
                xsb_s, negmu = xsbs[s], negmus[s]
                t2p = CVKV.tile([R, ST], F32, tag="cv")
                nc.tensor.matmul(t2p[:], avT[:, 0, :], xsb_s[:, 0, :], start=True, stop=False)
                nc.tensor.matmul(t2p[:], avT[:, 1, :], xsb_s[:, 1, :], start=False, stop=False)
                nc.tensor.matmul(t2p[:], avg1, negmu[:], start=False, stop=True)
                t2 = TMP.tile([R, ST], F16, tag="t2")
                nc.vector.tensor_copy(out=t2[:], in_=t2p[:])
                t2s.append(t2)
                for kvoc in range(2):
                    kps = CVKV.tile([P, ST], F32, tag="cv")
                    nc.tensor.matmul(kps[:], kvwT[:, 0, kvoc * P:(kvoc + 1) * P],
                                     xsb_s[:, 0, :], start=True, stop=False)
                    nc.tensor.matmul(kps[:], kvwT[:, 1, kvoc * P:(kvoc + 1) * P],
                                     xsb_s[:, 1, :], start=False, stop=False)
                    nc.tensor.matmul(kps[:], wg1k[:, kvoc, :], negmu[:],
                                     start=False, stop=False)
                    nc.tensor.matmul(kps[:], bvTk[:, kvoc, :], t2[:],
                                     start=False, stop=True)
                    if s == 0:
                        nc.scalar.copy(out=k8tmp[:, kvoc, s * ST:(s + 1) * ST],
                                       in_=kps[:])
                    else:
                        nc.vector.tensor_copy(
                            out=k8tmp[:, kvoc, s * ST:(s + 1) * ST], in_=kps[:])

            def k_repack(s):
                # odd heads h=2a+1 (packed base 16h=32a+16) -> kO base 32a
                for a in range(4):
                    h = 2 * a + 1
                    nc.gpsimd.dma_start(
                        out=kO[32 * a:32 * a + 16, :, s * ST:(s + 1) * ST],
                        in_=k8tmp[16 * h:16 * h + 16, :, s * ST:(s + 1) * ST])

            def v_path(s):
                # token-major v: out[m, vchan] via xsb/negmu/t2 as stationary
                xsb_s, negmu, t2, an_s = xsbs[s], negmus[s], t2s[s], ans[s]
                vsb_s = STR.tile([P, 4, HEAD, DH + 1], F16, tag="vsb")
                for ml in range(4):
                    sl = slice(ml * P, (ml + 1) * P)
                    vP = CVKV.tile([P, C], F32, tag="cv")
                    nc.tensor.matmul(vP[:], xsb_s[:, 0, sl], kvwT[:, 0, 256:512],
                                     start=True, stop=False)
                    nc.tensor.matmul(vP[:], xsb_s[:, 1, sl], kvwT[:, 1, 256:512],
                                     start=False, stop=False)
                    nc.tensor.matmul(vP[:], negmu[:, sl], wg1v,
                                     start=False, stop=False, tile_position=(0, 0))
                    nc.tensor.matmul(vP[:], t2[:, sl], bvTv,
                                     start=False, stop=True, tile_position=(0, 0))
                    nc.vector.tensor_scalar_mul(
                        out=vsb_s[:, ml, :, 0:DH],
                        in0=vP[:].rearrange("p (h d) -> p h d", d=DH),
                        scalar1=an_s[:, ml:ml + 1])
                nc.gpsimd.memset(vsb_s[:, :, :, DH:DH + 1], 1.0)
                vsbs.append(vsb_s)
                if DEBUG:
                    nc.sync.dma_start(out=dbg["d_vsb"][s], in_=vsb_s[:])

            def q_path():
                xs_t = xTs[0]
                tqp = P1.tile([R, NB], F32, tag="p1")
                for nh in range(2):
                    sl = slice(nh * 512, (nh + 1) * 512)
                    nc.tensor.matmul(tqp[:, sl], aqT[:, 0, :], xs_t[:, 0, sl],
                                     start=True, stop=False)
                    nc.tensor.matmul(tqp[:, sl], aqT[:, 1, :], xs_t[:, 1, sl],
                                     start=False, stop=True)
                nc.vector.tensor_copy(out=tq[:], in_=tqp[:])
                for oc in range(2):
                    qps = P1.tile([P, NB], F32, tag="p1")
                    for nh in range(2):
                        sl = slice(nh * 512, (nh + 1) * 512)
                        nc.tensor.matmul(qps[:, sl],
                                         qwT[:, 0, oc * P:(oc + 1) * P],
                                         xs_t[:, 0, sl], start=True, stop=False)
                        nc.tensor.matmul(qps[:, sl],
                                         qwT[:, 1, oc * P:(oc + 1) * P],
                                         xs_t[:, 1, sl], start=False, stop=False)
                        nc.tensor.matmul(qps[:, sl], bqT[:, oc, :], tq[:, sl],
                                         start=False, stop=True)
                    nc.vector.tensor_scalar_add(
                        out=q8tmp[:, oc, :], in0=qps[:], scalar1=wf[:, oc:oc + 1])
                for a in range(4):
                    h = 2 * a + 1
                    nc.sync.dma_start(
                        out=qO[32 * a:32 * a + 16],
                        in_=q8tmp[16 * h:16 * h + 16])

            # ---------------- attention pieces ----------------
            ptshs = [None] * HEAD

            def score_pair(h, mcp):
                # scores+exp for mc = 2*mcp, 2*mcp+1; even heads read the
                # packed tiles directly (base 16h is 32-aligned), odd heads
                # the repacked kO/qO
                if h % 2 == 0:
                    a, kT, qT8 = h // 2, k8tmp, q8tmp
                else:
                    a, kT, qT8 = h // 2, kO, qO
                if ptshs[h] is None:
                    ptsh = PTS.tile([P, 8, NB], F16, tag="ptsh")
                    ptshs[h] = ptsh
                ptsh = ptshs[h]
                for mc in (2 * mcp, 2 * mcp + 1):
                    s_, ml = mc // 4, mc % 4
                    stile = P1.tile([P, NB], F32, tag="p1")
                    for qc in range(4):
                        nc.tensor.matmul(
                            stile[:, qc * 256:(qc + 1) * 256],
                            kT[32 * a:32 * a + 16, :, mc * P:(mc + 1) * P],
                            qT8[32 * a:32 * a + 16, :, qc * 256:(qc + 1) * 256],
                            start=True, stop=True, perf_mode=PM.DoubleRow,
                            tile_position=(32 * a, 0))
                    if EXP_SCHED[h][mc] == "A":
                        nc.scalar.activation(out=ptsh[:, mc, :], in_=stile[:],
                                             func=AF.Exp,
                                             scale=ascls[s_][:, ml:ml + 1])
                    else:
                        nc.vector.tensor_scalar(
                            out=ptsh[:, mc, :].bitcast(I16), in0=stile[:],
                            scalar1=abits[s_][:, ml:ml + 1], scalar2=BITB,
                            op0=ALU.mult, op1=ALU.add)

            def pv_head(h):
                ptsh = ptshs[h]
                pvq = CVKV.tile([P, 8, DH + 1], F32, tag="cv")
                for qb in range(8):
                    for mc in range(8):
                        nc.tensor.matmul(
                            pvq[:, qb, :],
                            ptsh[:, mc, qb * P:(qb + 1) * P],
                            vsbs[mc // 4][:, mc % 4, h, :],
                            start=(mc == 0), stop=(mc == 7),
                            tile_position=(0, 0))
                rec = TMP.tile([P, 8], F32, tag="rec")
                nc.vector.reciprocal(out=rec[:], in_=pvq[:, :, DH])
                nc.vector.tensor_tensor(out=onorm[:, :, h, :],
                                        in0=pvq[:, :, 0:DH],
                                        in1=s0(rec[:], DH), op=ALU.mult)
                if DEBUG:
                    if h == 0:
                        nc.sync.dma_start(out=dbg["d_pts0"][:], in_=ptsh[:])
                    nc.sync.dma_start(out=dbg["d_rec"][h], in_=rec[:])

            def transpose_ch(ch):
                for qb in range(8):
                    ttile = CVKV.tile([P, P], F16, tag="cv")
                    nc.tensor.transpose(
                        ttile[:],
                        onorm[:, qb, ch * 4:(ch + 1) * 4, :].rearrange(
                            "p h d -> p (h d)"),
                        ident)
                    nc.vector.tensor_copy(
                        out=outT[:, ch, qb * P:(qb + 1) * P], in_=ttile[:])

            # ---------------- emission schedule ----------------
            conv_oc(0, 0)
            conv_oc(0, 1)
            stats(0)
            t2_and_k(0)
            k_repack(0)
            q_path()
            v_path(0)

            # interleave strip 1 with early scores (all heads, kv blocks 0-3)
            s1_chunks = [
                lambda: conv_oc(1, 0),
                lambda: conv_oc(1, 1),
                lambda: stats(1),
                lambda: t2_and_k(1),
                lambda: (k_repack(1), v_path(1)),
            ]
            b1_chunks = [(h, mcp) for h in (0, 2, 4, 6, 1, 3, 5, 7)
                         for mcp in (0, 1)]
            bi = 0
            for chunk in s1_chunks:
                chunk()
                for _ in range(2):
                    if bi < len(b1_chunks):
                        score_pair(*b1_chunks[bi])
                        bi += 1
            while bi < len(b1_chunks):
                score_pair(*b1_chunks[bi])
                bi += 1

            # second half: kv blocks 4-7 per head, pv pipelined one head behind
            for h in range(HEAD):
                score_pair(h, 2)
                score_pair(h, 3)
                if h > 0:
                    pv_head(h - 1)
                if h == 4:
                    transpose_ch(0)   # heads 0-3 normalized by now
            pv_head(HEAD - 1)
            if DEBUG:
                nc.sync.dma_start(out=dbg["d_onorm"][:], in_=onorm[:])
            transpose_ch(1)
            if DEBUG:
                nc.sync.dma_start(out=dbg["d_outT"][:], in_=outT[:])

            # ---------------- projection (per-t8, pipelined) ----------------
            for t8 in range(8):
                pp = P1.tile([P, C], F32, tag="p1")
                nc.tensor.matmul(pp[:], outT[:, 0, t8 * P:(t8 + 1) * P],
                                 pwT[:, 0, :], start=True, stop=False)
                nc.tensor.matmul(pp[:], outT[:, 1, t8 * P:(t8 + 1) * P],
                                 pwT[:, 1, :], start=False, stop=True)
                fin = FIN.tile([P, C], F32, tag="fin")
                nc.vector.tensor_add(out=fin[:], in0=pp[:], in1=pbB[:])
                nc.sync.dma_start(out=out_d[t8 * P:(t8 + 1) * P], in_=fin[:])

    nc.finalize()
    return nc


P_ = 128


def _kq_perm():
    # PSUM chunk oc holds half `oc` of all heads: partition p=16h+lane ->
    # original channel 32h + 16*oc + lane
    perm = np.empty(2 * P_, np.int64)
    for oc in range(2):
        for p in range(P_):
            perm[oc * P_ + p] = 32 * (p // 16) + 16 * oc + (p % 16)
    return perm


def _prep_shared(q_w, q_b, kv_w, kv_b, proj_w, proj_b, a_q, b_q, a_v, b_v,
                 sr_w, sr_b, ln_g, ln_b):
    f32 = np.float32

    def chunkT(w):  # [out, in] -> [128, n_in_chunks, out]
        wt = np.ascontiguousarray(np.asarray(w, f32).T)
        ic, oc = wt.shape
        return np.ascontiguousarray(
            wt.reshape(ic // 128, 128, oc).transpose(1, 0, 2)).astype(F16NP)

    kv_w = np.asarray(kv_w, f32)
    a_v = np.asarray(a_v, f32)
    b_v = np.asarray(b_v, f32)
    b_q = np.asarray(b_q, f32)
    q_w = np.asarray(q_w, f32)
    q_b = np.asarray(q_b, f32)
    g = np.asarray(ln_g, f32)
    bb = np.asarray(ln_b, f32)
    proj_w = np.asarray(proj_w, f32)

    # fold LayerNorm gamma into kv/a_v weights; mean via rank-1 correction;
    # k-side constants dropped (softmax shift invariance), v-side constants
    # folded into the projection bias.
    Wg = kv_w * g[None, :]
    wg1 = Wg.sum(1)
    Avg = a_v * g[None, :]
    avg1 = Avg.sum(1)
    wbt = kv_w @ bb + np.asarray(kv_b, f32)
    dconst = b_v @ (a_v @ bb)
    wv_const = wbt[C:] + dconst
    pb_eff = np.asarray(proj_b, f32) + proj_w @ wv_const

    perm = _kq_perm()
    qw_p = q_w[perm]
    qb_p = q_b[perm]
    bq_p = b_q[perm]
    Wgk_p = Wg[0:C][perm]
    wg1k_p = wg1[0:C][perm]
    bvk_p = b_v[perm]

    qwT = chunkT(qw_p)                                 # [128, 2, 256]
    kvwT = chunkT(np.concatenate([Wgk_p, Wg[C:]], 0))  # [128, 2, 512]
    pwT = chunkT(proj_w)
    srwT = np.asarray(sr_w, f32).transpose(1, 2, 3, 0).reshape(2, 128, 4, C)
    srwT = np.ascontiguousarray(srwT.transpose(1, 0, 2, 3)).astype(F16NP)
    aqT = chunkT(a_q)                                  # [128, 2, 8]
    avT = chunkT(Avg)
    bqT = np.ascontiguousarray(bq_p.T.reshape(R, 2, 128)).astype(F16NP)
    bvTk = np.ascontiguousarray(bvk_p.T.reshape(R, 2, 128)).astype(F16NP)

    wbc = np.zeros((128, WBC_COLS), F16NP)
    wbc[:, OC_SRWT:OC_SRWT + 2048] = srwT.reshape(128, 2048)
    wbc[:, OC_ONE1C] = F16NP(1.0 / C)

    blob = np.zeros((128, BLOB_COLS), F16NP)
    blob[:, O_QWT:O_QWT + 512] = qwT.reshape(128, 512)
    blob[:, O_KVWT:O_KVWT + 1024] = kvwT.reshape(128, 1024)
    blob[:, O_PWT:O_PWT + 512] = pwT.reshape(128, 512)
    blob[:, O_AQT:O_AQT + 16] = aqT.reshape(128, 16)
    blob[:, O_AVT:O_AVT + 16] = avT.reshape(128, 16)
    blob[:, O_IDENT:O_IDENT + 128] = np.eye(128, dtype=F16NP)
    blob[0:R, O_BQT:O_BQT + 256] = bqT.reshape(R, 256)
    blob[0:R, O_BVTK:O_BVTK + 256] = bvTk.reshape(R, 256)
    blob[0:R, O_BVTV:O_BVTV + 256] = b_v.T.astype(F16NP)
    blob[0, O_WG1K:O_WG1K + 256] = wg1k_p.reshape(2, 128).astype(F16NP).reshape(256)
    blob[0, O_WG1V:O_WG1V + 256] = wg1[C:].astype(F16NP)
    blob[0, O_AVG1:O_AVG1 + 8] = avg1.astype(F16NP)

    def pcols(v):  # [n*128] -> [128, n]
        v = np.asarray(v, f32)
        return np.ascontiguousarray(v.reshape(-1, 128).T)

    wf = np.zeros((128, 4), f32)
    wf[:, 0:2] = pcols(qb_p)
    wf[:, 2:4] = pcols(np.asarray(sr_b, f32))
    return dict(wbc=wbc, wb=blob, wf=wf, pb=pb_eff)


def kernel(x, q_w, q_b, kv_w, kv_b, proj_w, proj_b, a_q, b_q, a_v, b_v,
           sr_w, sr_b, ln_g, ln_b, H, W):
    from concourse.bass_utils import run_bass_kernel_spmd

    x = np.asarray(x, np.float32)
    assert x.shape == (B, N, C) and int(H) == 64 and int(W) == 64

    if "nc" not in _CACHE:
        _CACHE["nc"] = _build_program()
    nc = _CACHE["nc"]

    shared = _prep_shared(q_w, q_b, kv_w, kv_b, proj_w, proj_b, a_q, b_q,
                          a_v, b_v, sr_w, sr_b, ln_g, ln_b)
    in_maps = []
    for c in range(NCORES):
        b, j = c // 4, c % 4
        xb = np.roll(x[b], -NB * j, axis=0)             # own block at rows 0:1024
        xT = np.ascontiguousarray(xb.T.astype(F16NP))   # [256, 4096]
        xT = np.ascontiguousarray(
            xT.reshape(2, 128, N).transpose(1, 0, 2))   # [128, 2, 4096]
        in_maps.append(dict(shared, xT=xT))

    res = run_bass_kernel_spmd(nc, in_maps, list(range(NCORES)))
    _CACHE["res"] = res
    out = np.empty((B, N, C), np.float32)
    for c in range(NCORES):
        b, j = c // 4, c % 4
        out[b, NB * j:NB * (j + 1)] = res.results[c]["out"]
    return out


# revision 22
# speedup vs baseline: 1.6527x; 1.0016x over previous
"""Trainium2 Bass kernel for PVT-style spatial-reduction attention (v3).

Shapes (hardcoded): x [2, 4096, 256], HEAD=8, dh=32, SR=2, R=8, H=W=64.
Sharding: core c = (batch b = c//4, query block j = c%4). Each core computes
q/attention/proj for its 1024 query rows and redundantly computes the small
conv+LN+KV path for its batch. Per-core x is pre-rotated on host so each
core's own query block is rows 0:1024.

v3 design:
- fp16 compute chain (conv/kv/q/pv/proj); fp8e4m3 DoubleRow score matmuls
  (0.5 cyc/row) via host-permuted q/k projection columns + per-head repack
  DMAs to 32-aligned lane bases.
- Transposed pv (pts stationary): 33-row pv matmuls, per-partition softmax
  denominators (single reciprocal + stride-0 scale, no broadcasts).
- V computed token-major directly (xsb stationary) - no PE transposes.
- Softmax exp split across ScalarE (true Exp) and DVE (one-op Schraudolph
  int16 bit-trick into fp16); per-head schedule interleaves the engines.
- Attention starts after strip 0: scores for kv blocks 0-3 of all heads are
  emitted interleaved with strip-1 conv/kv so ScalarE/DVE start ~15us in.
- Few large DMAs; conv weights in a separate first blob for an early start.

PSUM (8 banks): P1 2x[128,1024]f32 (4) for q-path/scores/proj, CVKV
2x[128,512]f32 (2) for conv/kv/v/transposes, PV 2x[128,8,33]f32 (2) for pv.
"""
import sys

if "/opt/trn_rl_repo" not in sys.path:
    sys.path.insert(0, "/opt/trn_rl_repo")

import numpy as np

F16NP = np.float16

HEAD, DH, C, N, B, M, R = 8, 32, 256, 4096, 2, 1024, 8
NB = N // 4          # query rows per core
SCALE = DH ** -0.5
NCORES = 8
MAGIC = 0x5F3759DF
LOG2E8 = 1477.3195879  # 2^10 / ln 2
BITB = 15317.95        # tuned fp16 Schraudolph offset (trunc semantics)

# per-head exp engine schedule over mc=0..7 (A=ScalarE exp, D=DVE bit trick);
# alternating 6A2D / 5A3D heads -> 44 ACT / 20 DVE tiles.
EXP_SCHED = ["AADAADAA"] * 5 + ["ADAADAAD"] * 1 + ["ADADADAA"] * 2

# conv blob (wbc): srwT 2048 | ones1c 1
OC_SRWT, OC_ONE1C = 0, 2048
WBC_COLS = 2064
# main blob (wb) column offsets (fp16 elements)
O_QWT, O_KVWT, O_PWT = 0, 512, 1536
O_AQT, O_AVT, O_IDENT, O_BQT = 2048, 2064, 2080, 2208
O_BVTK, O_BVTV, O_WG1K, O_WG1V = 2464, 2720, 2976, 3232
O_AVG1, O_ONESR, O_PBROW = 3488, 3496, 3624
BLOB_COLS = 3880

_CACHE = {}
DEBUG = False


def _build_program():
    import concourse.bass as bass
    import concourse.tile as tile
    from concourse.bacc import Bacc
    from concourse import mybir

    F32 = mybir.dt.float32
    F16 = mybir.dt.float16
    FP8 = mybir.dt.float8e4
    I16 = mybir.dt.int16
    I32 = mybir.dt.int32
    U8 = mybir.dt.uint8
    AF = mybir.ActivationFunctionType
    ALU = mybir.AluOpType
    PM = mybir.MatmulPerfMode

    nc = Bacc()
    P = 128
    ST = 512       # kv tokens per strip

    def s0(ap, n):
        # stride-0 broadcast along a new innermost free dim
        return bass.AP(tensor=ap.tensor, offset=ap.offset,
                       ap=[list(d) for d in ap.ap] + [[0, n]])

    def bcast(ap, nparts):
        return bass.AP(tensor=ap.tensor, offset=ap.offset,
                       ap=[[0, nparts]] + [list(d) for d in ap.ap])

    xT_d = nc.declare_dram_parameter("xT", [P, 2, N], F16, isOutput=False)
    wbc_d = nc.declare_dram_parameter("wbc", [P, WBC_COLS], F16, isOutput=False)
    wb_d = nc.declare_dram_parameter("wb", [P, BLOB_COLS], F16, isOutput=False)
    wf_d = nc.declare_dram_parameter("wf", [P, 4], F32, isOutput=False)
    pb_d = nc.declare_dram_parameter("pb", [C], F32, isOutput=False)
    out_d = nc.declare_dram_parameter("out", [NB, C], F32, isOutput=True)
    if DEBUG:
        dbg = {
            "d_xsb": nc.declare_dram_parameter("d_xsb", [2, P, 2, 512], F16, isOutput=True),
            "d_an": nc.declare_dram_parameter("d_an", [2, P, 4], F32, isOutput=True),
            "d_k8": nc.declare_dram_parameter("d_k8", [P, 2, M], U8, isOutput=True),
            "d_q8": nc.declare_dram_parameter("d_q8", [P, 2, NB], U8, isOutput=True),
            "d_kA": nc.declare_dram_parameter("d_kA", [P, 2, M], U8, isOutput=True),
            "d_qA": nc.declare_dram_parameter("d_qA", [P, 2, NB], U8, isOutput=True),
            "d_vsb": nc.declare_dram_parameter("d_vsb", [2, P, 4, HEAD, DH + 1], F16, isOutput=True),
            "d_pts0": nc.declare_dram_parameter("d_pts0", [P, 8, NB], F16, isOutput=True),
            "d_rec": nc.declare_dram_parameter("d_rec", [HEAD, P, 8], F32, isOutput=True),
            "d_onorm": nc.declare_dram_parameter("d_onorm", [P, 8, HEAD, DH], F16, isOutput=True),
            "d_outT": nc.declare_dram_parameter("d_outT", [P, 2, NB], F16, isOutput=True),
        }

    with tile.TileContext(nc) as tc:
        with tc.tile_pool(name="wgt", bufs=1) as WGT, \
             tc.tile_pool(name="acts", bufs=1) as ACTS, \
             tc.tile_pool(name="str", bufs=2) as STR, \
             tc.tile_pool(name="tmp", bufs=2) as TMP, \
             tc.tile_pool(name="pts", bufs=8) as PTS, \
             tc.tile_pool(name="fin", bufs=8) as FIN, \
             tc.tile_pool(name="p1", bufs=3, space="PSUM") as P1, \
             tc.tile_pool(name="cvkv", bufs=2, space="PSUM") as CVKV, \
             tc.tile_pool(name="dscr", bufs=2, space="DRAM") as DSCR:

            # ---------------- loads (order matters for early start) ----------
            wbc = WGT.tile([P, WBC_COLS], F16, tag="wbc")
            nc.sync.dma_start(out=wbc[:], in_=wbc_d[:])
            wf = WGT.tile([P, 4], F32, tag="wf")
            nc.sync.dma_start(out=wf[:], in_=wf_d[:])
            xs0 = ACTS.tile([P, 2, 2048], F16, tag="xT0")
            nc.sync.dma_start(out=xs0[:], in_=xT_d[:, :, 0:2048])
            wb = WGT.tile([P, BLOB_COLS], F16, tag="wb")
            nc.sync.dma_start(out=wb[:], in_=wb_d[:])
            xs1 = ACTS.tile([P, 2, 2048], F16, tag="xT1")
            nc.sync.dma_start(out=xs1[:], in_=xT_d[:, :, 2048:4096])
            pbB = WGT.tile([P, C], F32, tag="pbB")
            nc.sync.dma_start(out=pbB[:], in_=bcast(pb_d.ap(), P))
            xTs = [xs0, xs1]

            srwT = wbc[:, OC_SRWT:OC_SRWT + 2048].rearrange(
                "p (ch t c) -> p ch t c", ch=2, t=4)
            ones1c = wbc[:, OC_ONE1C:OC_ONE1C + 1]
            qwT = wb[:, O_QWT:O_QWT + 512].rearrange("p (ch c) -> p ch c", ch=2)
            kvwT = wb[:, O_KVWT:O_KVWT + 1024].rearrange("p (ch c) -> p ch c", ch=2)
            pwT = wb[:, O_PWT:O_PWT + 512].rearrange("p (ch c) -> p ch c", ch=2)
            aqT = wb[:, O_AQT:O_AQT + 16].rearrange("p (ch r) -> p ch r", ch=2)
            avT = wb[:, O_AVT:O_AVT + 16].rearrange("p (ch r) -> p ch r", ch=2)
            ident = wb[:, O_IDENT:O_IDENT + 128]
            bqT = wb[0:R, O_BQT:O_BQT + 256].rearrange("r (ch c) -> r ch c", ch=2)
            bvTk = wb[0:R, O_BVTK:O_BVTK + 256].rearrange("r (ch c) -> r ch c", ch=2)
            bvTv = wb[0:R, O_BVTV:O_BVTV + 256]
            wg1k = wb[0:1, O_WG1K:O_WG1K + 256].rearrange("a (ch c) -> a ch c", ch=2)
            wg1v = wb[0:1, O_WG1V:O_WG1V + 256]
            avg1 = wb[0:1, O_AVG1:O_AVG1 + 8]
            onesr = wb[0:1, O_ONESR:O_ONESR + 128]
            pbrow = wb[0:1, O_PBROW:O_PBROW + 256]

            # persistent activations
            k8tmp = ACTS.tile([P, 2, M], FP8, tag="k8tmp")
            q8tmp = ACTS.tile([P, 2, NB], FP8, tag="q8tmp")
            kO = ACTS.tile([P, 2, M], FP8, tag="kO")
            qO = ACTS.tile([P, 2, NB], FP8, tag="qO")
            outT = ACTS.tile([P, 2, NB], F16, tag="outT")
            onorm = ACTS.tile([P, 8, HEAD, DH], F16, tag="onorm")
            tq = ACTS.tile([R, NB], F16, tag="tq")

            xsbs, sqs, negmus, t2s, vsbs, ans, ascls, abits = \
                [], [], [], [], [], [], [], []

            # ---------------- per-strip pieces as emission chunks ------------
            def conv_oc(s, oc):
                xs_t = xTs[s]
                if oc == 0:
                    xsb_s = STR.tile([P, 2, ST], F16, tag="xsb")
                    xsbs.append(xsb_s)
                xsb_s = xsbs[s]
                cps = CVKV.tile([P, ST], F32, tag="cv")
                first = True
                for cc in range(2):
                    xv = xs_t[:, cc, :].rearrange(
                        "p (i a j b) -> p i a j b", i=16, a=2, j=32, b=2)
                    for di in range(2):
                        for dj in range(2):
                            nc.tensor.matmul(
                                cps[:], srwT[:, cc, di * 2 + dj,
                                             oc * P:(oc + 1) * P],
                                xv[:, :, di, :, dj],
                                start=first,
                                stop=(cc == 1 and di == 1 and dj == 1))
                            first = False
                if s == 0:
                    nc.scalar.activation(out=xsb_s[:, oc, :], in_=cps[:],
                                         func=AF.Identity,
                                         bias=wf[:, 2 + oc:3 + oc])
                else:
                    nc.vector.tensor_scalar_add(
                        out=xsb_s[:, oc, :], in0=cps[:], scalar1=wf[:, 2 + oc:3 + oc])
                if DEBUG and oc == 1:
                    nc.sync.dma_start(out=dbg["d_xsb"][s], in_=xsb_s[:])

            def stats(s):
                xsb_s = xsbs[s]
                sq_s = STR.tile([P, 2, ST], F16, tag="sq")
                nc.gpsimd.tensor_mul(out=sq_s[:], in0=xsb_s[:], in1=xsb_s[:])
                sxp = CVKV.tile([1, ST], F32, tag="cv")
                nc.tensor.matmul(sxp[:], ones1c, xsb_s[:, 0, :], start=True, stop=False)
                nc.tensor.matmul(sxp[:], ones1c, xsb_s[:, 1, :], start=False, stop=True)
                negmu = TMP.tile([1, ST], F16, tag="negmu")
                nc.vector.tensor_scalar_mul(out=negmu[:], in0=sxp[:], scalar1=-1.0)
                negmus.append(negmu)
                sxxp = CVKV.tile([1, ST], F32, tag="cv")
                nc.tensor.matmul(sxxp[:], ones1c, sq_s[:, 0, :], start=True, stop=False)
                nc.tensor.matmul(sxxp[:], ones1c, sq_s[:, 1, :], start=False, stop=True)
                ex2_sb = TMP.tile([1, ST], F32, tag="ex2sb")
                nc.vector.tensor_copy(out=ex2_sb[:], in_=sxxp[:])

                # chunk-major repack [1,512] -> [128,4] via DRAM bounce
                nm_d = DSCR.tile([ST], F16, tag=f"nm{s}")
                nc.sync.dma_start(out=nm_d[:], in_=negmu[:])
                ex_d = DSCR.tile([ST], F32, tag=f"ex{s}")
                nc.sync.dma_start(out=ex_d[:], in_=ex2_sb[:])
                mur = TMP.tile([P, 4], F16, tag="mur")
                nc.sync.dma_start(out=mur[:],
                                  in_=nm_d[:].rearrange("(g p) -> p g", p=P))
                ex2r = TMP.tile([P, 4], F32, tag="ex2r")
                nc.sync.dma_start(out=ex2r[:],
                                  in_=ex_d[:].rearrange("(g p) -> p g", p=P))

                # rstd via quake rsqrt (1 newton), [128,4] chain on DVE
                nmu2 = TMP.tile([P, 4], F32, tag="nmu2")
                nc.vector.scalar_tensor_tensor(out=nmu2[:], in0=mur[:], scalar=-1.0,
                                               in1=mur[:], op0=ALU.mult, op1=ALU.mult)
                ve = TMP.tile([P, 4], F32, tag="ve")
                nc.vector.scalar_tensor_tensor(out=ve[:], in0=nmu2[:], scalar=1e-5,
                                               in1=ex2r[:], op0=ALU.add, op1=ALU.add)
                hsh = TMP.tile([P, 4], I32, tag="hsh")
                nc.vector.tensor_scalar(out=hsh[:], in0=ve[:].bitcast(I32), scalar1=1,
                                        scalar2=None, op0=ALU.logical_shift_right)
                nc.vector.tensor_scalar(out=hsh[:], in0=hsh[:], scalar1=-1,
                                        scalar2=MAGIC, op0=ALU.mult, op1=ALU.add)
                y0 = hsh[:].bitcast(F32)
                nt = TMP.tile([P, 4], F32, tag="nt")
                nc.vector.tensor_mul(out=nt[:], in0=y0, in1=y0)
                nc.vector.scalar_tensor_tensor(out=nt[:], in0=nt[:], scalar=-0.5,
                                               in1=ve[:], op0=ALU.mult, op1=ALU.mult)
                nc.vector.tensor_scalar_add(out=nt[:], in0=nt[:], scalar1=1.5)
                an_s = STR.tile([P, 4], F32, tag="an")
                nc.vector.tensor_mul(out=an_s[:], in0=y0, in1=nt[:])
                ascl_s = STR.tile([P, 4], F32, tag="ascl")
                nc.vector.tensor_scalar_mul(out=ascl_s[:], in0=an_s[:], scalar1=SCALE)
                abit_s = STR.tile([P, 4], F32, tag="abit")
                nc.vector.tensor_scalar_mul(out=abit_s[:], in0=ascl_s[:],
                                            scalar1=LOG2E8)
                ans.append(an_s)
                ascls.append(ascl_s)
                abits.append(abit_s)
                if DEBUG:
                    nc.sync.dma_start(out=dbg["d_an"][s], in_=an_s[:])

            def t2_and_k(s):
                xsb_s, negmu = xsbs[s], negmus[s]
                t2p = CVKV.tile([R, ST], F32, tag="cv")
                nc.tensor.matmul(t2p[:], avT[:, 0, :], xsb_s[:, 0, :], start=True, stop=False)
                nc.tensor.matmul(t2p[:], avT[:, 1, :], xsb_s[:, 1, :], start=False, stop=False)
                nc.tensor.matmul(t2p[:], avg1, negmu[:], start=False, stop=True)
                t2 = TMP.tile([R, ST], F16, tag="t2")
                nc.vector.tensor_copy(out=t2[:], in_=t2p[:])
                t2s.append(t2)
                for kvoc in range(2):
                    kps = CVKV.tile([P, ST], F32, tag="cv")
                    nc.tensor.matmul(kps[:], kvwT[:, 0, kvoc * P:(kvoc + 1) * P],
                                     xsb_s[:, 0, :], start=True, stop=False)
                    nc.tensor.matmul(kps[:], kvwT[:, 1, kvoc * P:(kvoc + 1) * P],
                                     xsb_s[:, 1, :], start=False, stop=False)
                    nc.tensor.matmul(kps[:], wg1k[:, kvoc, :], negmu[:],
                                     start=False, stop=False)
                    nc.tensor.matmul(kps[:], bvTk[:, kvoc, :], t2[:],
                                     start=False, stop=True)
                    if s == 0:
                        nc.scalar.copy(out=k8tmp[:, kvoc, s * ST:(s + 1) * ST],
                                       in_=kps[:])
                    else:
                        nc.vector.tensor_copy(
                            out=k8tmp[:, kvoc, s * ST:(s + 1) * ST], in_=kps[:])

            def k_repack(s):
                # odd heads h=2a+1 (packed base 16h=32a+16) -> kO base 32a
                for a in range(4):
                    h = 2 * a + 1
                    nc.gpsimd.dma_start(
                        out=kO[32 * a:32 * a + 16, :, s * ST:(s + 1) * ST],
                        in_=k8tmp[16 * h:16 * h + 16, :, s * ST:(s + 1) * ST])

            def v_path(s):
                # token-major v: out[m, vchan] via xsb/negmu/t2 as stationary
                xsb_s, negmu, t2, an_s = xsbs[s], negmus[s], t2s[s], ans[s]
                vsb_s = STR.tile([P, 4, HEAD, DH + 1], F16, tag="vsb")
                for ml in range(4):
                    sl = slice(ml * P, (ml + 1) * P)
                    vP = CVKV.tile([P, C], F32, tag="cv")
                    nc.tensor.matmul(vP[:], xsb_s[:, 0, sl], kvwT[:, 0, 256:512],
                                     start=True, stop=False)
                    nc.tensor.matmul(vP[:], xsb_s[:, 1, sl], kvwT[:, 1, 256:512],
                                     start=False, stop=False)
                    nc.tensor.matmul(vP[:], negmu[:, sl], wg1v,
                                     start=False, stop=False, tile_position=(0, 0))
                    nc.tensor.matmul(vP[:], t2[:, sl], bvTv,
                                     start=False, stop=True, tile_position=(0, 0))
                    nc.vector.tensor_scalar_mul(
                        out=vsb_s[:, ml, :, 0:DH],
                        in0=vP[:].rearrange("p (h d) -> p h d", d=DH),
                        scalar1=an_s[:, ml:ml + 1])
                nc.gpsimd.memset(vsb_s[:, :, :, DH:DH + 1], 1.0)
                vsbs.append(vsb_s)
                if DEBUG:
                    nc.sync.dma_start(out=dbg["d_vsb"][s], in_=vsb_s[:])

            def q_path():
                xs_t = xTs[0]
                tqp = P1.tile([R, NB], F32, tag="p1")
                for nh in range(2):
                    sl = slice(nh * 512, (nh + 1) * 512)
                    nc.tensor.matmul(tqp[:, sl], aqT[:, 0, :], xs_t[:, 0, sl],
                                     start=True, stop=False)
                    nc.tensor.matmul(tqp[:, sl], aqT[:, 1, :], xs_t[:, 1, sl],
                                     start=False, stop=True)
                nc.vector.tensor_copy(out=tq[:], in_=tqp[:])
                for oc in range(2):
                    qps = P1.tile([P, NB], F32, tag="p1")
                    for nh in range(2):
                        sl = slice(nh * 512, (nh + 1) * 512)
                        nc.tensor.matmul(qps[:, sl],
                                         qwT[:, 0, oc * P:(oc + 1) * P],
                                         xs_t[:, 0, sl], start=True, stop=False)
                        nc.tensor.matmul(qps[:, sl],
                                         qwT[:, 1, oc * P:(oc + 1) * P],
                                         xs_t[:, 1, sl], start=False, stop=False)
                        nc.tensor.matmul(qps[:, sl], bqT[:, oc, :], tq[:, sl],
                                         start=False, stop=True)
                    nc.vector.tensor_scalar_add(
                        out=q8tmp[:, oc, :], in0=qps[:], scalar1=wf[:, oc:oc + 1])
                for a in range(4):
                    h = 2 * a + 1
                    nc.sync.dma_start(
                        out=qO[32 * a:32 * a + 16],
                        in_=q8tmp[16 * h:16 * h + 16])

            # ---------------- attention pieces ----------------
            ptshs = [None] * HEAD

            def score_pair(h, mcp):
                # scores+exp for mc = 2*mcp, 2*mcp+1; even heads read the
                # packed tiles directly (base 16h is 32-aligned), odd heads
                # the repacked kO/qO
                if h % 2 == 0:
                    a, kT, qT8 = h // 2, k8tmp, q8tmp
                else:
                    a, kT, qT8 = h // 2, kO, qO
                if ptshs[h] is None:
                    ptsh = PTS.tile([P, 8, NB], F16, tag="ptsh")
                    ptshs[h] = ptsh
                ptsh = ptshs[h]
                for mc in (2 * mcp, 2 * mcp + 1):
                    s_, ml = mc // 4, mc % 4
                    stile = P1.tile([P, NB], F32, tag="p1")
                    for qc in range(4):
                        nc.tensor.matmul(
                            stile[:, qc * 256:(qc + 1) * 256],
                            kT[32 * a:32 * a + 16, :, mc * P:(mc + 1) * P],
                            qT8[32 * a:32 * a + 16, :, qc * 256:(qc + 1) * 256],
                            start=True, stop=True, perf_mode=PM.DoubleRow,
                            tile_position=(32 * a, 0))
                    if EXP_SCHED[h][mc] == "A":
                        nc.scalar.activation(out=ptsh[:, mc, :], in_=stile[:],
                                             func=AF.Exp,
                                             scale=ascls[s_][:, ml:ml + 1])
                    else:
                        nc.vector.tensor_scalar(
                            out=ptsh[:, mc, :].bitcast(I16), in0=stile[:],
                            scalar1=abits[s_][:, ml:ml + 1], scalar2=BITB,
                            op0=ALU.mult, op1=ALU.add)

            def pv_head(h):
                ptsh = ptshs[h]
                pvq = CVKV.tile([P, 8, DH + 1], F32, tag="cv")
                for qb in range(8):
                    for mc in range(8):
                        nc.tensor.matmul(
                            pvq[:, qb, :],
                            ptsh[:, mc, qb * P:(qb + 1) * P],
                            vsbs[mc // 4][:, mc % 4, h, :],
                            start=(mc == 0), stop=(mc == 7),
                            tile_position=(0, 0))
                rec = TMP.tile([P, 8], F32, tag="rec")
                nc.vector.reciprocal(out=rec[:], in_=pvq[:, :, DH])
                nc.vector.tensor_tensor(out=onorm[:, :, h, :],
                                        in0=pvq[:, :, 0:DH],
                                        in1=s0(rec[:], DH), op=ALU.mult)
                if DEBUG:
                    if h == 0:
                        nc.sync.dma_start(out=dbg["d_pts0"][:], in_=ptsh[:])
                    nc.sync.dma_start(out=dbg["d_rec"][h], in_=rec[:])

            def transpose_ch(ch):
                for qb in range(8):
                    ttile = CVKV.tile([P, P], F16, tag="cv")
                    nc.tensor.transpose(
                        ttile[:],
                        onorm[:, qb, ch * 4:(ch + 1) * 4, :].rearrange(
                            "p h d -> p (h d)"),
                        ident)
                    if ch == 1:
                        nc.scalar.copy(out=outT[:, ch, qb * P:(qb + 1) * P],
                                       in_=ttile[:])
                    else:
                        nc.vector.tensor_copy(
                            out=outT[:, ch, qb * P:(qb + 1) * P], in_=ttile[:])

            # ---------------- emission schedule ----------------
            conv_oc(0, 0)
            conv_oc(0, 1)
            stats(0)
            t2_and_k(0)
            k_repack(0)
            q_path()
            v_path(0)

            # interleave strip 1 with early scores (all heads, kv blocks 0-3)
            s1_chunks = [
                lambda: conv_oc(1, 0),
                lambda: conv_oc(1, 1),
                lambda: stats(1),
                lambda: t2_and_k(1),
                lambda: (k_repack(1), v_path(1)),
            ]
            b1_chunks = [(h, mcp) for h in (0, 2, 4, 6, 1, 3, 5, 7)
                         for mcp in (0, 1)]
            bi = 0
            for chunk in s1_chunks:
                chunk()
                for _ in range(2):
                    if bi < len(b1_chunks):
                        score_pair(*b1_chunks[bi])
                        bi += 1
            while bi < len(b1_chunks):
                score_pair(*b1_chunks[bi])
                bi += 1

            # second half: kv blocks 4-7 per head, pv pipelined one head behind
            for h in range(HEAD):
                score_pair(h, 2)
                score_pair(h, 3)
                if h > 0:
                    pv_head(h - 1)
                if h == 4:
                    transpose_ch(0)   # heads 0-3 normalized by now
            pv_head(HEAD - 1)
            if DEBUG:
                nc.sync.dma_start(out=dbg["d_onorm"][:], in_=onorm[:])
            transpose_ch(1)
            if DEBUG:
                nc.sync.dma_start(out=dbg["d_outT"][:], in_=outT[:])

            # ---------------- projection (per-t8, pipelined) ----------------
            for t8 in range(8):
                pp = P1.tile([P, C], F32, tag="p1")
                nc.tensor.matmul(pp[:], outT[:, 0, t8 * P:(t8 + 1) * P],
                                 pwT[:, 0, :], start=True, stop=False)
                nc.tensor.matmul(pp[:], outT[:, 1, t8 * P:(t8 + 1) * P],
                                 pwT[:, 1, :], start=False, stop=False)
                nc.tensor.matmul(pp[:], onesr, pbrow,
                                 start=False, stop=True, tile_position=(0, 0))
                fin = FIN.tile([P, C], F32, tag="fin")
                nc.scalar.copy(out=fin[:], in_=pp[:])
                nc.sync.dma_start(out=out_d[t8 * P:(t8 + 1) * P], in_=fin[:])

    nc.finalize()
    return nc


P_ = 128


def _kq_perm():
    # PSUM chunk oc holds half `oc` of all heads: partition p=16h+lane ->
    # original channel 32h + 16*oc + lane
    perm = np.empty(2 * P_, np.int64)
    for oc in range(2):
        for p in range(P_):
            perm[oc * P_ + p] = 32 * (p // 16) + 16 * oc + (p % 16)
    return perm


def _prep_shared(q_w, q_b, kv_w, kv_b, proj_w, proj_b, a_q, b_q, a_v, b_v,
                 sr_w, sr_b, ln_g, ln_b):
    f32 = np.float32

    def chunkT(w):  # [out, in] -> [128, n_in_chunks, out]
        wt = np.ascontiguousarray(np.asarray(w, f32).T)
        ic, oc = wt.shape
        return np.ascontiguousarray(
            wt.reshape(ic // 128, 128, oc).transpose(1, 0, 2)).astype(F16NP)

    kv_w = np.asarray(kv_w, f32)
    a_v = np.asarray(a_v, f32)
    b_v = np.asarray(b_v, f32)
    b_q = np.asarray(b_q, f32)
    q_w = np.asarray(q_w, f32)
    q_b = np.asarray(q_b, f32)
    g = np.asarray(ln_g, f32)
    bb = np.asarray(ln_b, f32)
    proj_w = np.asarray(proj_w, f32)

    # fold LayerNorm gamma into kv/a_v weights; mean via rank-1 correction;
    # k-side constants dropped (softmax shift invariance), v-side constants
    # folded into the projection bias.
    Wg = kv_w * g[None, :]
    wg1 = Wg.sum(1)
    Avg = a_v * g[None, :]
    avg1 = Avg.sum(1)
    wbt = kv_w @ bb + np.asarray(kv_b, f32)
    dconst = b_v @ (a_v @ bb)
    wv_const = wbt[C:] + dconst
    pb_eff = np.asarray(proj_b, f32) + proj_w @ wv_const

    perm = _kq_perm()
    qw_p = q_w[perm]
    qb_p = q_b[perm]
    bq_p = b_q[perm]
    Wgk_p = Wg[0:C][perm]
    wg1k_p = wg1[0:C][perm]
    bvk_p = b_v[perm]

    qwT = chunkT(qw_p)                                 # [128, 2, 256]
    kvwT = chunkT(np.concatenate([Wgk_p, Wg[C:]], 0))  # [128, 2, 512]
    pwT = chunkT(proj_w)
    srwT = np.asarray(sr_w, f32).transpose(1, 2, 3, 0).reshape(2, 128, 4, C)
    srwT = np.ascontiguousarray(srwT.transpose(1, 0, 2, 3)).astype(F16NP)
    aqT = chunkT(a_q)                                  # [128, 2, 8]
    avT = chunkT(Avg)
    bqT = np.ascontiguousarray(bq_p.T.reshape(R, 2, 128)).astype(F16NP)
    bvTk = np.ascontiguousarray(bvk_p.T.reshape(R, 2, 128)).astype(F16NP)

    wbc = np.zeros((128, WBC_COLS), F16NP)
    wbc[:, OC_SRWT:OC_SRWT + 2048] = srwT.reshape(128, 2048)
    wbc[:, OC_ONE1C] = F16NP(1.0 / C)

    blob = np.zeros((128, BLOB_COLS), F16NP)
    blob[:, O_QWT:O_QWT + 512] = qwT.reshape(128, 512)
    blob[:, O_KVWT:O_KVWT + 1024] = kvwT.reshape(128, 1024)
    blob[:, O_PWT:O_PWT + 512] = pwT.reshape(128, 512)
    blob[:, O_AQT:O_AQT + 16] = aqT.reshape(128, 16)
    blob[:, O_AVT:O_AVT + 16] = avT.reshape(128, 16)
    blob[:, O_IDENT:O_IDENT + 128] = np.eye(128, dtype=F16NP)
    blob[0:R, O_BQT:O_BQT + 256] = bqT.reshape(R, 256)
    blob[0:R, O_BVTK:O_BVTK + 256] = bvTk.reshape(R, 256)
    blob[0:R, O_BVTV:O_BVTV + 256] = b_v.T.astype(F16NP)
    blob[0, O_WG1K:O_WG1K + 256] = wg1k_p.reshape(2, 128).astype(F16NP).reshape(256)
    blob[0, O_WG1V:O_WG1V + 256] = wg1[C:].astype(F16NP)
    blob[0, O_AVG1:O_AVG1 + 8] = avg1.astype(F16NP)
    blob[0, O_ONESR:O_ONESR + 128] = F16NP(1.0)
    blob[0, O_PBROW:O_PBROW + 256] = pb_eff.astype(F16NP)

    def pcols(v):  # [n*128] -> [128, n]
        v = np.asarray(v, f32)
        return np.ascontiguousarray(v.reshape(-1, 128).T)

    wf = np.zeros((128, 4), f32)
    wf[:, 0:2] = pcols(qb_p)
    wf[:, 2:4] = pcols(np.asarray(sr_b, f32))
    return dict(wbc=wbc, wb=blob, wf=wf, pb=pb_eff)


def kernel(x, q_w, q_b, kv_w, kv_b, proj_w, proj_b, a_q, b_q, a_v, b_v,
           sr_w, sr_b, ln_g, ln_b, H, W):
    from concourse.bass_utils import run_bass_kernel_spmd

    x = np.asarray(x, np.float32)
    assert x.shape == (B, N, C) and int(H) == 64 and int(W) == 64

    if "nc" not in _CACHE:
        _CACHE["nc"] = _build_program()
    nc = _CACHE["nc"]

    shared = _prep_shared(q_w, q_b, kv_w, kv_b, proj_w, proj_b, a_q, b_q,
                          a_v, b_v, sr_w, sr_b, ln_g, ln_b)
    in_maps = []
    for c in range(NCORES):
        b, j = c // 4, c % 4
        xb = np.roll(x[b], -NB * j, axis=0)             # own block at rows 0:1024
        xT = np.ascontiguousarray(xb.T.astype(F16NP))   # [256, 4096]
        xT = np.ascontiguousarray(
            xT.reshape(2, 128, N).transpose(1, 0, 2))   # [128, 2, 4096]
        in_maps.append(dict(shared, xT=xT))

    res = run_bass_kernel_spmd(nc, in_maps, list(range(NCORES)))
    _CACHE["res"] = res
    out = np.empty((B, N, C), np.float32)
    for c in range(NCORES):
        b, j = c // 4, c % 4
        out[b, NB * j:NB * (j + 1)] = res.results[c]["out"]
    return out
